# revision 42
# baseline (speedup 1.0000x reference)
"""Attention-LSTM decoder (B=32, T=1000, S=100, D=512, A=1024, H=1024,
E=640, V=10240, P=1024) on 8 trn2 NeuronCores.

Sharding: data-parallel over batch, 4 batches per core (one per "slot").
Batches are sorted by enc_seq_len; slot j holds ranks [j*8:(j+1)*8] so the
padded time extent Tp[j] (multiple of 128) is shared by all 8 cores and the
SPMD graph is identical across cores.

v2 design (vs. baseline):
  - gate weights W_comb / W_s stored fp8e4m3 (x64 scale), moving operands
    h / ctx quantized to fp8 (x16); descale folded into activation scales.
  - LSTM sigmoids computed as 0.5*tanh(0.5x)+0.5 so the whole kernel uses
    one ACT table (exp_and_others: tanh+exp+identity) -- zero table loads
    inside the step loop.
  - attention z-pass = one STT per (slot, a-chunk) over the full Tp extent,
    alternating DVE / Pool engines; tanh on ACT with bias=s (per-partition).
  - energies accumulate into one PSUM tile [128,1024] at partition rows
    {0,32,64,96}; single Exp over the tile, multiplicative {0,1} mask,
    single reduce/recip/scale -- softmax is 1 ACT + 4 DVE ops total.
  - accum/finv/mask/w_att all live in the stride-32 row layout.
  - enc streamed per step as 4 large DMAs ([128, TC*512] per slot);
    q broadcast as one [128, 4096] DMA from a [1,4096] DRAM bounce.
  - ctx matvecs enc-stationary as before; w_att transposed via full
    128x128 PE transposes + stride-32 free-dim gather copies.
"""
import sys

sys.path.insert(0, "/opt/trn_rl_repo")

import os
import numpy as np
import ml_dtypes
from contextlib import ExitStack

import concourse.bass as bass
import concourse.tile as tile
import concourse.mybir as mybir
from concourse import bacc
from concourse.masks import make_identity

DT = mybir.dt
F32 = DT.float32
BF16 = DT.bfloat16
FP8 = DT.float8e4
AF = mybir.ActivationFunctionType
ALU = mybir.AluOpType
ET = mybir.EngineType

B, T, S = 32, 1000, 100
D, A, H, E, V, RO = 512, 1024, 1024, 640, 10240, 1024
ZH, ZC = 0.05, 0.15
NCORE = 8
BL = B // NCORE          # 4 batches (slots) per core
NS = S * BL              # 400 step-batch columns
GC = 4 * H // 128        # 32 gate chunks
HC = H // 128            # 8
AC = A // 128            # 8
DC = D // 128            # 4
EC = E // 128            # 5
ROC = RO // 2 // 128     # 4 chunks per maxout half
VC = V // 128            # 80 vocab chunks
XROC = (H + E + D) // 128  # 17 readout K-chunks

USE_FP8 = os.environ.get("KBFP8", "1") != "0"
SW = 64.0 if USE_FP8 else 1.0    # weight scale
SX = 16.0 if USE_FP8 else 1.0    # moving (h/ctx) scale
SG = SW * SX                      # psum scale for gates/s
WDT = FP8 if USE_FP8 else BF16

bf16 = ml_dtypes.bfloat16
f8 = ml_dtypes.float8_e4m3
LAST_EXEC_NS = None
LAST_OUTS = None
LAST_META = None


def _bf(a):
    return np.ascontiguousarray(np.asarray(a, dtype=np.float32)).astype(bf16)


def _w8(a):
    a = np.asarray(a, dtype=np.float32) * SW
    return np.ascontiguousarray(a).astype(f8 if USE_FP8 else bf16)


# gate-permutation: reference gate order is [i|f|g|o]; we reorder rows to
# [i|f|o|g] so the three sigmoids are contiguous.
def _gate_perm():
    idx = np.arange(4 * H)
    return np.concatenate([idx[0:2 * H], idx[3 * H:4 * H], idx[2 * H:3 * H]])


def build_nc(Tp, debug=False):
    TC = [t // 128 for t in Tp]
    TCmax = max(TC)
    nc = bacc.Bacc("TRN2", target_bir_lowering=False)

    def param(name, shape, dt=BF16):
        return nc.declare_dram_parameter(name, list(shape), dt, isOutput=False)

    enc_td = [param(f"enc_td{j}", [Tp[j], D]) for j in range(BL)]
    encT = [param(f"encT{j}", [D, Tp[j]]) for j in range(BL)]
    embT_d = param("embT", [E, NS])
    W_combT_d = param("W_combT", [D + H, 4 * H], WDT)
    W_ih_embT_d = param("W_ih_embT", [E, 4 * H])
    W_encT_d = param("W_encT", [D, A])
    W_sT_d = param("W_sT", [H, A], WDT)
    wfert_col_d = param("wfert_col", [128, DC])
    vT_col_d = param("vT_col", [128, AC])
    wfb_colf_d = param("wfb_colf", [128, AC], F32)
    b_enc_col_d = param("b_enc_col", [128, AC], F32)
    b_comb_d = param("b_comb", [128, GC], F32)     # pre-scaled by SG on host
    mask01_d = param("mask01", [BL, 1024])         # {0,1} rows
    W_roT_e_d = param("W_roT_e", [H + E + D, RO // 2])
    W_roT_o_d = param("W_roT_o", [H + E + D, RO // 2])
    b_ro_e_d = param("b_ro_e", [128, ROC], F32)
    b_ro_o_d = param("b_ro_o", [128, ROC], F32)
    W_outT_d = param("W_outT", [RO // 2, V])
    b_out_d = param("b_out_col", [128, VC], F32)
    out_d = nc.declare_dram_parameter("out", [V, NS], F32, isOutput=True)

    qd = nc.dram_tensor("qd", [1, BL * 1024], BF16)
    hstk_d = nc.dram_tensor("hstk", [H, NS], BF16)
    cstk_d = nc.dram_tensor("cstk", [D, NS], BF16)
    xembT_d = nc.dram_tensor("xembT", [4 * H, NS], BF16)

    with ExitStack() as ctx:
        tc = ctx.enter_context(tile.TileContext(nc))

        # ---------------- persistent pools ----------------
        persist = ctx.enter_context(tc.tile_pool(name="persist", bufs=1))
        ident = persist.tile([128, 128], BF16)
        make_identity(nc, ident[:])
        vT_col = persist.tile([128, AC], BF16)
        nc.sync.dma_start(vT_col[:], vT_col_d[:, :])
        wfb_colf = persist.tile([128, AC], F32)
        nc.sync.dma_start(wfb_colf[:], wfb_colf_d[:, :])
        wfert_col = persist.tile([128, DC], BF16)
        nc.sync.dma_start(wfert_col[:], wfert_col_d[:, :])
        b_enc_col = persist.tile([128, AC], F32)
        nc.sync.dma_start(b_enc_col[:], b_enc_col_d[:, :])
        b_comb = persist.tile([128, GC], F32)
        nc.sync.dma_start(b_comb[:], b_comb_d[:, :])
        mask01 = persist.tile([128, 1024], BF16)
        nc.vector.memset(mask01[:], 0.0)
        for j in range(BL):
            nc.sync.dma_start(mask01[32 * j:32 * j + 1, :], mask01_d[j:j + 1, :])

        h_bf = persist.tile([128, HC * BL], BF16)
        h_q = persist.tile([128, HC * BL], WDT)
        c_st = persist.tile([128, HC * BL], F32)
        ctxT_sb = persist.tile([128, DC * BL], BF16)
        ctx_q = persist.tile([128, DC * BL], WDT)
        s_sb = persist.tile([128, AC * BL], F32)
        accum_bf = persist.tile([128, 1024], BF16)  # rows {0,32,64,96}
        w_att = persist.tile([128, 1024], BF16)
        finv = persist.tile([128, 1024], BF16)     # rows {0,32,64,96}, x0.5
        wts = persist.tile([128, TCmax * BL], BF16)
        for t_ in (h_bf, h_q, c_st, ctxT_sb, ctx_q, s_sb, accum_bf,
                   w_att, finv, wts):
            nc.vector.memset(t_[:], 0.0)

        inner = ctx.enter_context(ExitStack())
        e_pool = inner.enter_context(tc.tile_pool(name="e", bufs=1))
        e_t = [[e_pool.tile([128, Tp[j]], BF16, name=f"e_{j}_{a}", tag=f"e{j}{a}")
                for a in range(AC)] for j in range(BL)]

        ppsum = ctx.enter_context(tc.tile_pool(name="ppsum", bufs=1, space="PSUM"))
        G = ppsum.tile([128, GC * BL], F32)
        Ghh = ppsum.tile([128, GC * BL], F32)
        trash_ps = ppsum.tile([128, 128], BF16)

        def pe_touch(ap):
            if ap.dtype not in (BF16,):
                return
            p = ap.shape[0]
            nc.tensor.transpose(trash_ps[0:min(ap.shape[1], 128), 0:p],
                                ap[:, 0:min(ap.shape[1], 128)], ident[0:p, 0:p])

        # ============ PRECOMPUTE PHASE ============
        with ExitStack() as pre:
            pre_sb = pre.enter_context(tc.tile_pool(name="pre_sb", bufs=1))
            pre_st = pre.enter_context(tc.tile_pool(name="pre_st", bufs=2))
            pre_ps = pre.enter_context(tc.tile_pool(name="pre_ps", bufs=1,
                                                    space="PSUM"))

            W_encT = [pre_sb.tile([128, A], BF16, name=f"wenc{k}", tag=f"we{k}")
                      for k in range(DC)]
            for k in range(DC):
                nc.sync.dma_start(W_encT[k][:], W_encT_d[k * 128:(k + 1) * 128, :])
            pe_touch(W_encT[0][:, 0:128])

            for j in range(BL):
                ercs = [pre_st.tile([128, Tp[j]], BF16, name=f"erc{j}{k}",
                                    tag=f"erc{k}") for k in range(DC)]
                for k in range(DC):
                    nc.sync.dma_start(ercs[k][:], encT[j][k * 128:(k + 1) * 128, :])
                    pe_touch(ercs[k][:, 0:128])
                for a in range(AC):
                    pe2 = pre_ps.tile([128, 1024], F32, name="pe_e2", tag="pe_e2")
                    for k in range(DC):
                        for n0 in range(0, Tp[j], 512):
                            n1 = min(n0 + 512, Tp[j])
                            nc.tensor.matmul(pe2[:, n0:n1],
                                             W_encT[k][:, a * 128:(a + 1) * 128],
                                             ercs[k][:, n0:n1],
                                             start=(k == 0), stop=(k == DC - 1))
                    nc.scalar.activation(e_t[j][a][:], pe2[:, 0:Tp[j]],
                                         AF.Identity,
                                         bias=b_enc_col[:, a:a + 1], scale=1.0)
                pf = pre_ps.tile([1, 1024], F32, name="pf", tag="pf")
                for k in range(DC):
                    for n0 in range(0, Tp[j], 512):
                        n1 = min(n0 + 512, Tp[j])
                        nc.tensor.matmul(pf[0:1, n0:n1], wfert_col[:, k:k + 1],
                                         ercs[k][:, n0:n1],
                                         start=(k == 0), stop=(k == DC - 1))
                # finv = 0.5*sigmoid(x) = 0.25*tanh(0.5x) + 0.25  (no table sw)
                fstage = pre_st.tile([1, 1024], F32, name="fstage", tag="fstage")
                nc.scalar.activation(fstage[0:1, 0:Tp[j]], pf[0:1, 0:Tp[j]],
                                     AF.Tanh, scale=0.5)
                fst2 = pre_st.tile([1, 1024], BF16, name="fst2", tag="fst2")
                nc.vector.tensor_scalar(fst2[0:1, 0:Tp[j]], fstage[0:1, 0:Tp[j]],
                                        0.25, 0.25, ALU.mult, ALU.add)
                nc.sync.dma_start(finv[32 * j:32 * j + 1, 0:Tp[j]],
                                  fst2[0:1, 0:Tp[j]])

            embT_sb = [pre_sb.tile([128, NS], BF16, name=f"embs{k}", tag=f"em{k}")
                       for k in range(EC)]
            for k in range(EC):
                nc.sync.dma_start(embT_sb[k][:], embT_d[k * 128:(k + 1) * 128, :])
            W_ie = [pre_sb.tile([128, 4 * H], BF16, name=f"wie{k}", tag=f"wi{k}")
                    for k in range(EC)]
            for k in range(EC):
                nc.sync.dma_start(W_ie[k][:], W_ih_embT_d[k * 128:(k + 1) * 128, :])
            pe_touch(W_ie[0][:, 0:128])
            pe_touch(embT_sb[0][:, 0:128])
            for g in range(GC):
                px = pre_ps.tile([128, NS], F32, name="px", tag="pe_e2")
                for k in range(EC):
                    nc.tensor.matmul(px[:], W_ie[k][:, g * 128:(g + 1) * 128],
                                     embT_sb[k][:], start=(k == 0),
                                     stop=(k == EC - 1))
                # xemb scaled by SG, bias pre-scaled on host
                stg = pre_st.tile([128, NS], BF16, name="xstg", tag="xstg")
                nc.scalar.activation(stg[:], px[:], AF.Identity,
                                     bias=b_comb[:, g:g + 1], scale=SG)
                nc.sync.dma_start(xembT_d[g * 128:(g + 1) * 128, :], stg[:])

        # ============ WEIGHTS (loop phase) ============
        w_pool = inner.enter_context(tc.tile_pool(name="w", bufs=1))
        W_comb = [w_pool.tile([128, 4 * H], WDT, name=f"wc{k}", tag=f"wc{k}")
                  for k in range(DC + HC)]
        for k in range(DC + HC):
            nc.sync.dma_start(W_comb[k][:], W_combT_d[k * 128:(k + 1) * 128, :])
        W_sT = [w_pool.tile([128, A], WDT, name=f"ws{k}", tag=f"ws{k}")
                for k in range(HC)]
        for k in range(HC):
            nc.sync.dma_start(W_sT[k][:], W_sT_d[k * 128:(k + 1) * 128, :])

        loop_sb = inner.enter_context(tc.tile_pool(name="lsb", bufs=2))
        loop_z = inner.enter_context(tc.tile_pool(name="lz", bufs=3))
        loop_q = inner.enter_context(tc.tile_pool(name="lq", bufs=2))
        enc_pool = inner.enter_context(tc.tile_pool(name="encp", bufs=1))
        loop_ps = inner.enter_context(tc.tile_pool(name="lps", bufs=1, space="PSUM"))
        loop_ps2 = inner.enter_context(tc.tile_pool(name="lps2", bufs=1,
                                                    space="PSUM"))

        # enc resident in SBUF for the whole loop (identical every step)
        enc_all = []
        for j in range(BL):
            ea = enc_pool.tile([128, TC[j] * 512], BF16, name=f"ea{j}",
                               tag=f"ea{j}")
            nc.sync.dma_start(
                ea[:], enc_td[j][:, :].rearrange("(c p) d -> p c d", p=128))
            enc_all.append(ea)

        # prologue: W_hh part of step 0 with h=0 (initializes psum groups)
        pe_touch(W_comb[DC][:, 0:128])
        pe_touch(h_q[:, 0:HC * BL])
        for g in range(GC):
            for k in range(HC):
                nc.tensor.matmul(Ghh[:, g * BL:(g + 1) * BL],
                                 W_comb[DC + k][:, g * 128:(g + 1) * 128],
                                 h_q[:, k * BL:(k + 1) * BL],
                                 start=(k == 0), stop=(k == HC - 1),
                                 skip_group_check=True)
        # qd init (accum_bf is zeroed)
        for j in range(BL):
            nc.sync.dma_start(qd[0:1, j * 1024:(j + 1) * 1024],
                              accum_bf[32 * j:32 * j + 1, :])

        # ============ STEP LOOP ============
        _nsteps = int(os.environ.get("KBSTEPS", S))
        UNROLL = int(os.environ.get("KBUNROLL", 4))
        # Persistent energy PSUM tile, zeroed once: columns beyond Tp[j] are
        # never matmul-written, so they must hold 0 (exp(0)*mask0 = 0), not
        # garbage (exp(NaN/huge) would poison the softmax row sums).
        Eps = loop_ps.tile([128, 1024], F32, name="Eps", tag="Eps")
        nc.vector.memset(Eps[:], 0.0)

        def step_body(t4):
            # ---- head DMAs ----
            xet = loop_sb.tile([128, GC * BL], BF16, name="xet", tag="xet")
            src = xembT_d[:, bass.ds(t4, BL)].rearrange("(c p) b -> p c b", p=128)
            nc.sync.dma_start(xet[:], src)
            qbc = loop_q.tile([128, BL * 1024], BF16, name="qbc", tag="qbc")
            nc.sync.dma_start(qbc[:], qd[0:1, :].partition_broadcast(128))

            # ---- gates: ctx part ----
            for g in range(GC):
                for k in range(DC):
                    nc.tensor.matmul(G[:, g * BL:(g + 1) * BL],
                                     W_comb[k][:, g * 128:(g + 1) * 128],
                                     ctx_q[:, k * BL:(k + 1) * BL],
                                     start=(k == 0), stop=(k == DC - 1),
                                     skip_group_check=True)

            gates_f = loop_sb.tile([128, GC * BL], F32, name="gates_f", tag="gf")
            nc.vector.tensor_tensor(gates_f[:], G[:], xet[:], ALU.add)
            nc.vector.tensor_tensor(gates_f[:], gates_f[:], Ghh[:], ALU.add)

            # ifo chunks 0..23 -> cols 0:96 ; g chunks 24..31 -> cols 96:128
            t_ifo = loop_sb.tile([128, 3 * HC * BL], F32, name="t_ifo", tag="ti")
            nc.scalar.activation(t_ifo[:], gates_f[:, 0:3 * HC * BL], AF.Tanh,
                                 scale=0.5 / SG)
            t_g = loop_sb.tile([128, HC * BL], F32, name="t_g", tag="tg")
            nc.scalar.activation(t_g[:], gates_f[:, 3 * HC * BL:], AF.Tanh,
                                 scale=1.0 / SG)

            HB = HC * BL
            sl_i, sl_f, sl_o = (t_ifo[:, 0:HB], t_ifo[:, HB:2 * HB],
                                t_ifo[:, 2 * HB:3 * HB])
            sig_i = loop_sb.tile([128, HB], F32, name="sig_i", tag="si")
            nc.vector.tensor_scalar(sig_i[:], sl_i, 0.5, 0.5, ALU.mult, ALU.add)
            sig_f = loop_sb.tile([128, HB], F32, name="sig_f", tag="sf")
            nc.vector.tensor_scalar(sig_f[:], sl_f, 0.5, 0.5, ALU.mult, ALU.add)
            ig = loop_sb.tile([128, HB], F32, name="ig", tag="ig")
            nc.vector.tensor_tensor(ig[:], sig_i[:], t_g[:], ALU.mult)
            fc = loop_sb.tile([128, HB], F32, name="fc", tag="fcx")
            nc.vector.tensor_tensor(fc[:], sig_f[:], c_st[:], ALU.mult)
            cn = loop_sb.tile([128, HB], F32, name="cn", tag="cn")
            nc.vector.tensor_tensor(cn[:], fc[:], ig[:], ALU.add)
            tcn = loop_sb.tile([128, HB], F32, name="tcn", tag="tcn")
            nc.scalar.activation(tcn[:], cn[:], AF.Tanh)
            c05 = loop_sb.tile([128, HB], F32, name="c05", tag="c05")
            nc.vector.tensor_scalar(c05[:], c_st[:], ZC, None, ALU.mult)
            nc.vector.scalar_tensor_tensor(c_st[:], cn[:], 1.0 - ZC, c05[:],
                                           ALU.mult, ALU.add)
            sig_o = loop_sb.tile([128, HB], F32, name="sig_o", tag="so")
            nc.vector.tensor_scalar(sig_o[:], sl_o, 0.5, 0.5, ALU.mult, ALU.add)
            ot = loop_sb.tile([128, HB], F32, name="ot", tag="ot")
            nc.vector.tensor_tensor(ot[:], sig_o[:], tcn[:], ALU.mult)
            h05 = loop_sb.tile([128, HB], F32, name="h05", tag="h05")
            nc.vector.tensor_scalar(h05[:], h_bf[:], ZH, None, ALU.mult)
            nc.vector.scalar_tensor_tensor(h_bf[:], ot[:], 1.0 - ZH, h05[:],
                                           ALU.mult, ALU.add)
            nc.vector.tensor_scalar(h_q[:], h_bf[:], SX, None, ALU.mult)

            dst = hstk_d[:, bass.ds(t4, BL)].rearrange("(c p) b -> p c b", p=128)
            nc.sync.dma_start(dst, h_bf[:])

            # ---- s_t ----
            s_ps = loop_ps.tile([128, AC * BL], F32, name="s_ps", tag="s_ps")
            for a in range(AC):
                for k in range(HC):
                    nc.tensor.matmul(s_ps[:, a * BL:(a + 1) * BL],
                                     W_sT[k][:, a * 128:(a + 1) * 128],
                                     h_q[:, k * BL:(k + 1) * BL],
                                     start=(k == 0), stop=(k == HC - 1),
                                     skip_group_check=True)
            nc.vector.tensor_scalar(s_sb[:], s_ps[:], 1.0 / SG, None, ALU.mult)

            # ---- W_hh part of NEXT step's gates (first half; second half
            # is emitted after the attention matvecs to keep PE dense
            # through the softmax window) ----
            for g in range(GC // 2):
                for k in range(HC):
                    nc.tensor.matmul(Ghh[:, g * BL:(g + 1) * BL],
                                     W_comb[DC + k][:, g * 128:(g + 1) * 128],
                                     h_q[:, k * BL:(k + 1) * BL],
                                     start=(k == 0), stop=(k == HC - 1),
                                     skip_group_check=True)

            # ---- attention ----
            for j in range(BL):
                for a in range(AC):
                    ai = a * BL + j
                    # e_t holds e/wfb (host pre-divided); z = q + e/wfb,
                    # tanh applies scale=wfb, bias=s.
                    z = loop_z.tile([128, 1024], BF16, name="z", tag="z")
                    nc.vector.tensor_tensor(
                        z[:, 0:Tp[j]], qbc[:, j * 1024:j * 1024 + Tp[j]],
                        e_t[j][a][:], ALU.add)
                    tv = loop_z.tile([128, 1024], BF16, name="tv", tag="tv")
                    nc.scalar.activation(tv[:, 0:Tp[j]], z[:, 0:Tp[j]], AF.Tanh,
                                         bias=s_sb[:, ai:ai + 1],
                                         scale=wfb_colf[:, a:a + 1])
                    kw = dict(start=(a == 0), stop=(a == AC - 1),
                              skip_group_check=True)
                    if j:
                        kw["tile_position"] = (0, 32 * j)
                    nc.tensor.matmul(Eps[32 * j:32 * j + 1, 0:512],
                                     vT_col[:, a:a + 1], tv[:, 0:512], **kw)
                    if Tp[j] > 512:
                        nc.tensor.matmul(Eps[32 * j:32 * j + 1, 512:Tp[j]],
                                         vT_col[:, a:a + 1], tv[:, 512:Tp[j]],
                                         **kw)

            # ---- W_hh second half (fills the PE softmax window) ----
            for g in range(GC // 2, GC):
                for k in range(HC):
                    nc.tensor.matmul(Ghh[:, g * BL:(g + 1) * BL],
                                     W_comb[DC + k][:, g * 128:(g + 1) * 128],
                                     h_q[:, k * BL:(k + 1) * BL],
                                     start=(k == 0), stop=(k == HC - 1),
                                     skip_group_check=True)

            # ---- softmax (rows {0,32,64,96}) ----
            En = loop_sb.tile([128, 1024], BF16, name="En", tag="En")
            nc.scalar.activation(En[:], Eps[:], AF.Exp)
            w1 = loop_sb.tile([128, 1024], BF16, name="w1", tag="w1")
            nc.vector.tensor_tensor(w1[:], En[:], mask01[:], ALU.mult)
            se = loop_sb.tile([128, 1], F32, name="se", tag="se")
            nc.vector.tensor_reduce(se[:], w1[:], mybir.AxisListType.X, ALU.add)
            rse = loop_sb.tile([128, 1], F32, name="rse", tag="rse")
            nc.vector.reciprocal(rse[:], se[:])
            nc.vector.tensor_scalar(w_att[:], w1[:], rse[:], None, ALU.mult)

            # ---- accum += w*finv (bf16) ; qd bounce ----
            wf = loop_sb.tile([128, 1024], BF16, name="wf", tag="wf")
            nc.vector.tensor_tensor(wf[:], w_att[:], finv[:], ALU.mult)
            nc.vector.tensor_tensor(accum_bf[:], accum_bf[:], wf[:], ALU.add)
            for j in range(BL):
                nc.sync.dma_start(qd[0:1, j * 1024:(j + 1) * 1024],
                                  accum_bf[32 * j:32 * j + 1, :])

            # ---- transpose w_att -> wts [128,(tc,b)] ----
            for t in range(TCmax):
                wtp = loop_ps2.tile([128, 128], BF16, name="wtp", tag="wtp")
                nc.tensor.transpose(wtp[:, :], w_att[:, t * 128:(t + 1) * 128],
                                    ident[:, :])
                nc.vector.tensor_copy(wts[:, t * BL:(t + 1) * BL],
                                      wtp[:, 0:128:32])

            # ---- ctx matvecs (enc-stationary) ----
            c_ps = loop_ps.tile([128, DC * BL], F32, name="c_ps", tag="c_ps")
            for j in range(BL):
                for dk in range(DC):
                    for t in range(TC[j]):
                        nc.tensor.matmul(
                            c_ps[:, dk * BL + j:dk * BL + j + 1],
                            enc_all[j][:, t * 512 + dk * 128:t * 512 + (dk + 1) * 128],
                            wts[:, t * BL + j:t * BL + j + 1],
                            start=(t == 0), stop=(t == TC[j] - 1),
                            skip_group_check=True)
            nc.vector.tensor_copy(ctxT_sb[:], c_ps[:])
            nc.vector.tensor_scalar(ctx_q[:], c_ps[:], SX, None, ALU.mult)
            dst = cstk_d[:, bass.ds(t4, BL)].rearrange("(c p) b -> p c b", p=128)
            nc.sync.dma_start(dst, ctxT_sb[:])

        with tc.For_i(0, _nsteps * BL, UNROLL * BL,
                      hint_engines=(ET.PE, ET.Activation, ET.DVE, ET.SP)) as t4:
            for s in range(UNROLL):
                step_body(t4 + s * BL)

        # ============ READOUT ============
        inner.close()
        post_sb = ctx.enter_context(tc.tile_pool(name="post_sb", bufs=1))
        post_st = ctx.enter_context(tc.tile_pool(name="post_st", bufs=2))
        post_ps = ctx.enter_context(tc.tile_pool(name="post_ps", bufs=2,
                                                 space="PSUM"))

        xro = []
        for k in range(HC):
            tl = post_sb.tile([128, NS], BF16, name=f"xh{k}", tag=f"xh{k}")
            nc.sync.dma_start(tl[:], hstk_d[k * 128:(k + 1) * 128, :])
            xro.append(tl)
        for k in range(EC):
            tl = post_sb.tile([128, NS], BF16, name=f"xe{k}", tag=f"xe{k}")
            nc.sync.dma_start(tl[:], embT_d[k * 128:(k + 1) * 128, :])
            xro.append(tl)
        for k in range(DC):
            tl = post_sb.tile([128, NS], BF16, name=f"xc{k}", tag=f"xc{k}")
            nc.sync.dma_start(tl[:], cstk_d[k * 128:(k + 1) * 128, :])
            xro.append(tl)
        W_roe = [post_sb.tile([128, RO // 2], BF16, name=f"wre{k}", tag=f"wre{k}")
                 for k in range(XROC)]
        W_roo = [post_sb.tile([128, RO // 2], BF16, name=f"wro{k}", tag=f"wro{k}")
                 for k in range(XROC)]
        for k in range(XROC):
            nc.sync.dma_start(W_roe[k][:], W_roT_e_d[k * 128:(k + 1) * 128, :])
            nc.sync.dma_start(W_roo[k][:], W_roT_o_d[k * 128:(k + 1) * 128, :])
        b_ro_e = post_sb.tile([128, ROC], F32)
        nc.sync.dma_start(b_ro_e[:], b_ro_e_d[:, :])
        b_ro_o = post_sb.tile([128, ROC], F32)
        nc.sync.dma_start(b_ro_o[:], b_ro_o_d[:, :])
        b_out_col = post_sb.tile([128, VC], F32)
        nc.sync.dma_start(b_out_col[:], b_out_d[:, :])
        pe_touch(xro[0][:, 0:128])
        pe_touch(W_roe[0][:, 0:128])
        pe_touch(W_roo[0][:, 0:128])

        maxo = []
        for oc in range(ROC):
            Re = post_ps.tile([128, NS], F32, name="Re", tag="Re")
            for k in range(XROC):
                nc.tensor.matmul(Re[:], W_roe[k][:, oc * 128:(oc + 1) * 128],
                                 xro[k][:], start=(k == 0), stop=(k == XROC - 1))
            t1 = post_st.tile([128, NS], F32, name="t1", tag="t1")
            nc.scalar.activation(t1[:], Re[:], AF.Identity,
                                 bias=b_ro_e[:, oc:oc + 1], scale=1.0)
            Ro = post_ps.tile([128, NS], F32, name="Ro", tag="Re")
            for k in range(XROC):
                nc.tensor.matmul(Ro[:], W_roo[k][:, oc * 128:(oc + 1) * 128],
                                 xro[k][:], start=(k == 0), stop=(k == XROC - 1))
            t2 = post_st.tile([128, NS], F32, name="t2", tag="t2")
            nc.scalar.activation(t2[:], Ro[:], AF.Identity,
                                 bias=b_ro_o[:, oc:oc + 1], scale=1.0)
            mo = post_sb.tile([128, NS], BF16, name=f"mo{oc}", tag=f"mo{oc}")
            nc.vector.tensor_tensor(mo[:], t1[:], t2[:], ALU.max)
            maxo.append(mo)

        wo_pool = ctx.enter_context(tc.tile_pool(name="wo", bufs=6))
        first = True
        for vc in range(VC):
            wo = [wo_pool.tile([128, 128], BF16, name=f"wo{vc}_{k}", tag=f"wok{k}")
                  for k in range(ROC)]
            for k in range(ROC):
                nc.sync.dma_start(wo[k][:],
                                  W_outT_d[k * 128:(k + 1) * 128,
                                           vc * 128:(vc + 1) * 128])
            if first:
                pe_touch(wo[0][:, 0:128])
                pe_touch(maxo[0][:, 0:128])
                first = False
            L = post_ps.tile([128, NS], F32, name="L", tag="L")
            for k in range(ROC):
                nc.tensor.matmul(L[:], wo[k][:], maxo[k][:],
                                 start=(k == 0), stop=(k == ROC - 1))
            lo = post_st.tile([128, NS], F32, name="lo", tag="lo")
            nc.scalar.activation(lo[:], L[:], AF.Identity,
                                 bias=b_out_col[:, vc:vc + 1], scale=1.0)
            nc.sync.dma_start(out_d[vc * 128:(vc + 1) * 128, :], lo[:])

    return nc


def check_waits(nc, cap_note=""):
    bad = []
    for fn in nc.m.functions:
        for bb in fn.blocks:
            for inst in bb.instructions:
                c = inst.concise()
                nw = c.count("wait:")
                eng = c.split()[0] if c.split() else "?"
                if nw >= 2 and eng in ("PE", "ACT", "DVE", "PL"):
                    bad.append((nw, c[:180]))
    for nw, c in bad:
        print("WAITS", nw, c)
    return bad


def _prep_core(inputs, order, Tp, core):
    enc = np.asarray(inputs["encoder_outputs"], np.float32)
    labels = np.asarray(inputs["labels"])
    lens = np.asarray(inputs["enc_seq_len"], np.int64)
    embed = np.asarray(inputs["embed"], np.float32)

    bidx = [int(order[j * NCORE + core]) for j in range(BL)]
    m = {}
    for j in range(BL):
        b = bidx[j]
        ep = np.zeros((Tp[j], D), np.float32)
        ep[:T] = enc[b, :Tp[j] if Tp[j] <= T else T]
        m[f"enc_td{j}"] = _bf(ep)
        m[f"encT{j}"] = _bf(ep.T)
    emb = np.zeros((BL, S, E), np.float32)
    for j in range(BL):
        b = bidx[j]
        emb[j, 1:] = embed[labels[b, :S - 1].astype(np.int64)]
    embT = emb.transpose(2, 1, 0).reshape(E, NS)
    m["embT"] = _bf(embT)
    mask01 = np.zeros((BL, 1024), np.float32)
    for j in range(BL):
        mask01[j, :int(lens[bidx[j]])] = 1.0
    m["mask01"] = _bf(mask01)
    return m, bidx


def kernel(**inputs):
    lens = np.asarray(inputs["enc_seq_len"], np.int64)
    order = np.argsort(-lens, kind="stable")
    Tp = []
    for j in range(BL):
        mx = max(int(lens[order[j * NCORE + i]]) for i in range(NCORE))
        Tp.append(min(1024, ((mx + 127) // 128) * 128))

    perm = _gate_perm()
    W_ih = np.asarray(inputs["W_ih"], np.float32)[perm]
    W_hh = np.asarray(inputs["W_hh"], np.float32)[perm]
    b_sum = (np.asarray(inputs["b_ih"], np.float32)
             + np.asarray(inputs["b_hh"], np.float32))[perm]
    wfb = np.asarray(inputs["W_fb"], np.float32)[:, 0]
    wfb_safe = np.where(wfb >= 0, np.maximum(wfb, 1e-3),
                        np.minimum(wfb, -1e-3))
    shared = {
        "W_combT": _w8(np.concatenate([W_ih[:, E:].T, W_hh.T], 0)),
        "W_ih_embT": _bf(W_ih[:, :E].T),
        "W_encT": _bf(np.asarray(inputs["W_enc"], np.float32).T
                      / wfb_safe[None, :]),
        "W_sT": _w8(np.asarray(inputs["W_s"], np.float32).T),
        "wfert_col": _bf(np.asarray(inputs["W_fert"],
                                    np.float32).reshape(DC, 128).T),
        "vT_col": _bf(np.asarray(inputs["v_att"], np.float32).reshape(AC, 128).T),
        "wfb_colf": np.ascontiguousarray(
            wfb_safe.reshape(AC, 128).T.astype(np.float32)),
        "b_enc_col": np.ascontiguousarray(
            (np.asarray(inputs["b_enc"], np.float32) / wfb_safe)
            .reshape(AC, 128).T),
        "b_comb": np.ascontiguousarray(
            (b_sum * SG).reshape(GC, 128).T),
        "W_roT_e": _bf(np.asarray(inputs["W_ro"], np.float32)[0::2].T),
        "W_roT_o": _bf(np.asarray(inputs["W_ro"], np.float32)[1::2].T),
        "b_ro_e": np.ascontiguousarray(
            np.asarray(inputs["b_ro"], np.float32)[0::2].reshape(ROC, 128).T),
        "b_ro_o": np.ascontiguousarray(
            np.asarray(inputs["b_ro"], np.float32)[1::2].reshape(ROC, 128).T),
        "W_outT": _bf(np.asarray(inputs["W_out"], np.float32).T),
        "b_out_col": np.ascontiguousarray(
            np.asarray(inputs["b_out"], np.float32).reshape(VC, 128).T),
    }

    in_maps = []
    bidx_all = []
    for c in range(NCORE):
        m, bidx = _prep_core(inputs, order, Tp, c)
        m.update(shared)
        in_maps.append(m)
        bidx_all.append(bidx)

    nc = build_nc(Tp)
    nc.finalize()
    from concourse.bass_utils import run_bass_kernel_spmd
    trace = bool(os.environ.get("BASS_KERNEL_TRACE"))
    res = run_bass_kernel_spmd(nc, in_maps, core_ids=list(range(NCORE)),
                               trace=trace)
    global LAST_EXEC_NS, LAST_OUTS, LAST_META
    LAST_EXEC_NS = res.exec_time_ns
    outs = res.results
    LAST_OUTS = outs
    LAST_META = (order, Tp, bidx_all)

    logits = np.zeros((B, S, V), np.float32)
    for c in range(NCORE):
        o = outs[c]["out"].reshape(V, S, BL)
        for j in range(BL):
            logits[bidx_all[c][j]] = o[:, :, j].T
    return logits


if __name__ == "__main__":
    nc = build_nc([1024, 896, 768, 640])
    bad = check_waits(nc)
    print(f"{len(bad)} instructions with >=2 waits")


# revision 44
# speedup vs baseline: 1.1870x; 1.1870x over previous
"""Attention-LSTM decoder (B=32, T=1000, S=100, D=512, A=1024, H=1024,
E=640, V=10240, P=1024) on 8 trn2 NeuronCores.

Sharding: data-parallel over batch, 4 batches per core (one per "slot").
Batches are sorted by enc_seq_len; slot j holds ranks [j*8:(j+1)*8] so the
padded time extent Tp[j] (multiple of 128) is shared by all 8 cores and the
SPMD graph is identical across cores.

v2 design (vs. baseline):
  - gate weights W_comb / W_s stored fp8e4m3 (x64 scale), moving operands
    h / ctx quantized to fp8 (x16); descale folded into activation scales.
  - LSTM sigmoids computed as 0.5*tanh(0.5x)+0.5 so the whole kernel uses
    one ACT table (exp_and_others: tanh+exp+identity) -- zero table loads
    inside the step loop.
  - attention z-pass = one STT per (slot, a-chunk) over the full Tp extent,
    alternating DVE / Pool engines; tanh on ACT with bias=s (per-partition).
  - energies accumulate into one PSUM tile [128,1024] at partition rows
    {0,32,64,96}; single Exp over the tile, multiplicative {0,1} mask,
    single reduce/recip/scale -- softmax is 1 ACT + 4 DVE ops total.
  - accum/finv/mask/w_att all live in the stride-32 row layout.
  - enc streamed per step as 4 large DMAs ([128, TC*512] per slot);
    q broadcast as one [128, 4096] DMA from a [1,4096] DRAM bounce.
  - ctx matvecs enc-stationary as before; w_att transposed via full
    128x128 PE transposes + stride-32 free-dim gather copies.
"""
import sys

sys.path.insert(0, "/opt/trn_rl_repo")

import os
import numpy as np
import ml_dtypes
from contextlib import ExitStack

import concourse.bass as bass
import concourse.tile as tile
import concourse.mybir as mybir
from concourse import bacc
from concourse.masks import make_identity

DT = mybir.dt
F32 = DT.float32
BF16 = DT.bfloat16
FP8 = DT.float8e4
AF = mybir.ActivationFunctionType
ALU = mybir.AluOpType
ET = mybir.EngineType

B, T, S = 32, 1000, 100
D, A, H, E, V, RO = 512, 1024, 1024, 640, 10240, 1024
ZH, ZC = 0.05, 0.15
NCORE = 8
BL = B // NCORE          # 4 batches (slots) per core
NS = S * BL              # 400 step-batch columns
GC = 4 * H // 128        # 32 gate chunks
HC = H // 128            # 8
AC = A // 128            # 8
DC = D // 128            # 4
EC = E // 128            # 5
ROC = RO // 2 // 128     # 4 chunks per maxout half
VC = V // 128            # 80 vocab chunks
XROC = (H + E + D) // 128  # 17 readout K-chunks

USE_FP8 = os.environ.get("KBFP8", "1") != "0"
SW = 64.0 if USE_FP8 else 1.0    # weight scale
SX = 16.0 if USE_FP8 else 1.0    # moving (h/ctx) scale
SG = SW * SX                      # psum scale for gates/s
WDT = FP8 if USE_FP8 else BF16

bf16 = ml_dtypes.bfloat16
f8 = ml_dtypes.float8_e4m3
LAST_EXEC_NS = None
LAST_OUTS = None
LAST_META = None


def _bf(a):
    return np.ascontiguousarray(np.asarray(a, dtype=np.float32)).astype(bf16)


def _w8(a):
    a = np.asarray(a, dtype=np.float32) * SW
    return np.ascontiguousarray(a).astype(f8 if USE_FP8 else bf16)


# gate-permutation: reference gate order is [i|f|g|o]; we reorder rows to
# [i|f|o|g] so the three sigmoids are contiguous.
def _gate_perm():
    idx = np.arange(4 * H)
    return np.concatenate([idx[0:2 * H], idx[3 * H:4 * H], idx[2 * H:3 * H]])


def build_nc(Tp, debug=False):
    TC = [t // 128 for t in Tp]
    TCmax = max(TC)
    nc = bacc.Bacc("TRN2", target_bir_lowering=False)

    def param(name, shape, dt=BF16):
        return nc.declare_dram_parameter(name, list(shape), dt, isOutput=False)

    enc_td = [param(f"enc_td{j}", [Tp[j], D]) for j in range(BL)]
    encT = [param(f"encT{j}", [D, Tp[j]]) for j in range(BL)]
    embT_d = param("embT", [E, NS])
    W_combT_d = param("W_combT", [D + H, 4 * H], WDT)
    W_ih_embT_d = param("W_ih_embT", [E, 4 * H])
    W_encT_d = param("W_encT", [D, A])
    W_sT_d = param("W_sT", [H, A], WDT)
    wfert_col_d = param("wfert_col", [128, DC])
    vT_col_d = param("vT_col", [128, AC])
    wfb_colf_d = param("wfb_colf", [128, AC], F32)
    b_enc_col_d = param("b_enc_col", [128, AC], F32)
    b_comb_d = param("b_comb", [128, GC], F32)     # pre-scaled by SG on host
    mask01_d = param("mask01", [BL, 1024])         # {0,1} rows
    W_roT_e_d = param("W_roT_e", [H + E + D, RO // 2])
    W_roT_o_d = param("W_roT_o", [H + E + D, RO // 2])
    b_ro_e_d = param("b_ro_e", [128, ROC], F32)
    b_ro_o_d = param("b_ro_o", [128, ROC], F32)
    W_outT_d = param("W_outT", [RO // 2, V])
    b_out_d = param("b_out_col", [128, VC], F32)
    out_d = nc.declare_dram_parameter("out", [V, NS], F32, isOutput=True)

    qd = nc.dram_tensor("qd", [1, BL * 1024], BF16)
    hstk_d = nc.dram_tensor("hstk", [H, NS], BF16)
    cstk_d = nc.dram_tensor("cstk", [D, NS], BF16)
    xembT_d = nc.dram_tensor("xembT", [4 * H, NS], BF16)

    with ExitStack() as ctx:
        tc = ctx.enter_context(tile.TileContext(nc))

        # ---------------- persistent pools ----------------
        persist = ctx.enter_context(tc.tile_pool(name="persist", bufs=1))
        ident = persist.tile([128, 128], BF16)
        make_identity(nc, ident[:])
        vT_col = persist.tile([128, AC], BF16)
        nc.sync.dma_start(vT_col[:], vT_col_d[:, :])
        wfb_colf = persist.tile([128, AC], F32)
        nc.sync.dma_start(wfb_colf[:], wfb_colf_d[:, :])
        wfert_col = persist.tile([128, DC], BF16)
        nc.sync.dma_start(wfert_col[:], wfert_col_d[:, :])
        b_enc_col = persist.tile([128, AC], F32)
        nc.sync.dma_start(b_enc_col[:], b_enc_col_d[:, :])
        b_comb = persist.tile([128, GC], F32)
        nc.sync.dma_start(b_comb[:], b_comb_d[:, :])
        mask01 = persist.tile([128, 1024], BF16)
        nc.vector.memset(mask01[:], 0.0)
        for j in range(BL):
            nc.sync.dma_start(mask01[32 * j:32 * j + 1, :], mask01_d[j:j + 1, :])

        h_bf = persist.tile([128, HC * BL], BF16)
        h_q = persist.tile([128, HC * BL], WDT)
        c_st = persist.tile([128, HC * BL], F32)
        ctxT_sb = persist.tile([128, DC * BL], BF16)
        ctx_q = persist.tile([128, DC * BL], WDT)
        s_sb = persist.tile([128, AC * BL], F32)
        accum_bf = persist.tile([128, 1024], BF16)  # rows {0,32,64,96}
        w_att = persist.tile([128, 1024], BF16)
        finv = persist.tile([128, 1024], BF16)     # rows {0,32,64,96}, x0.5
        wts = persist.tile([128, TCmax * BL], BF16)
        for t_ in (h_bf, h_q, c_st, ctxT_sb, ctx_q, s_sb, accum_bf,
                   w_att, finv, wts):
            nc.vector.memset(t_[:], 0.0)

        inner = ctx.enter_context(ExitStack())
        e_pool = inner.enter_context(tc.tile_pool(name="e", bufs=1))
        e_t = [[e_pool.tile([128, Tp[j]], BF16, name=f"e_{j}_{a}", tag=f"e{j}{a}")
                for a in range(AC)] for j in range(BL)]

        ppsum = ctx.enter_context(tc.tile_pool(name="ppsum", bufs=1, space="PSUM"))
        G = ppsum.tile([128, GC * BL], F32)
        Ghh = ppsum.tile([128, GC * BL], F32)
        trash_ps = ppsum.tile([128, 128], BF16)

        def pe_touch(ap):
            if ap.dtype not in (BF16,):
                return
            p = ap.shape[0]
            nc.tensor.transpose(trash_ps[0:min(ap.shape[1], 128), 0:p],
                                ap[:, 0:min(ap.shape[1], 128)], ident[0:p, 0:p])

        # ============ PRECOMPUTE PHASE ============
        with ExitStack() as pre:
            pre_sb = pre.enter_context(tc.tile_pool(name="pre_sb", bufs=1))
            pre_st = pre.enter_context(tc.tile_pool(name="pre_st", bufs=2))
            pre_ps = pre.enter_context(tc.tile_pool(name="pre_ps", bufs=1,
                                                    space="PSUM"))

            W_encT = [pre_sb.tile([128, A], BF16, name=f"wenc{k}", tag=f"we{k}")
                      for k in range(DC)]
            for k in range(DC):
                nc.sync.dma_start(W_encT[k][:], W_encT_d[k * 128:(k + 1) * 128, :])
            pe_touch(W_encT[0][:, 0:128])

            for j in range(BL):
                ercs = [pre_st.tile([128, Tp[j]], BF16, name=f"erc{j}{k}",
                                    tag=f"erc{k}") for k in range(DC)]
                for k in range(DC):
                    nc.sync.dma_start(ercs[k][:], encT[j][k * 128:(k + 1) * 128, :])
                    pe_touch(ercs[k][:, 0:128])
                for a in range(AC):
                    pe2 = pre_ps.tile([128, 1024], F32, name="pe_e2", tag="pe_e2")
                    for k in range(DC):
                        for n0 in range(0, Tp[j], 512):
                            n1 = min(n0 + 512, Tp[j])
                            nc.tensor.matmul(pe2[:, n0:n1],
                                             W_encT[k][:, a * 128:(a + 1) * 128],
                                             ercs[k][:, n0:n1],
                                             start=(k == 0), stop=(k == DC - 1))
                    nc.scalar.activation(e_t[j][a][:], pe2[:, 0:Tp[j]],
                                         AF.Identity,
                                         bias=b_enc_col[:, a:a + 1], scale=1.0)
                pf = pre_ps.tile([1, 1024], F32, name="pf", tag="pf")
                for k in range(DC):
                    for n0 in range(0, Tp[j], 512):
                        n1 = min(n0 + 512, Tp[j])
                        nc.tensor.matmul(pf[0:1, n0:n1], wfert_col[:, k:k + 1],
                                         ercs[k][:, n0:n1],
                                         start=(k == 0), stop=(k == DC - 1))
                # finv = 0.5*sigmoid(x) = 0.25*tanh(0.5x) + 0.25  (no table sw)
                fstage = pre_st.tile([1, 1024], F32, name="fstage", tag="fstage")
                nc.scalar.activation(fstage[0:1, 0:Tp[j]], pf[0:1, 0:Tp[j]],
                                     AF.Tanh, scale=0.5)
                fst2 = pre_st.tile([1, 1024], BF16, name="fst2", tag="fst2")
                nc.vector.tensor_scalar(fst2[0:1, 0:Tp[j]], fstage[0:1, 0:Tp[j]],
                                        0.25, 0.25, ALU.mult, ALU.add)
                nc.sync.dma_start(finv[32 * j:32 * j + 1, 0:Tp[j]],
                                  fst2[0:1, 0:Tp[j]])

            embT_sb = [pre_sb.tile([128, NS], BF16, name=f"embs{k}", tag=f"em{k}")
                       for k in range(EC)]
            for k in range(EC):
                nc.sync.dma_start(embT_sb[k][:], embT_d[k * 128:(k + 1) * 128, :])
            W_ie = [pre_sb.tile([128, 4 * H], BF16, name=f"wie{k}", tag=f"wi{k}")
                    for k in range(EC)]
            for k in range(EC):
                nc.sync.dma_start(W_ie[k][:], W_ih_embT_d[k * 128:(k + 1) * 128, :])
            pe_touch(W_ie[0][:, 0:128])
            pe_touch(embT_sb[0][:, 0:128])
            for g in range(GC):
                px = pre_ps.tile([128, NS], F32, name="px", tag="pe_e2")
                for k in range(EC):
                    nc.tensor.matmul(px[:], W_ie[k][:, g * 128:(g + 1) * 128],
                                     embT_sb[k][:], start=(k == 0),
                                     stop=(k == EC - 1))
                # xemb scaled by SG, bias pre-scaled on host
                stg = pre_st.tile([128, NS], BF16, name="xstg", tag="xstg")
                nc.scalar.activation(stg[:], px[:], AF.Identity,
                                     bias=b_comb[:, g:g + 1], scale=SG)
                nc.sync.dma_start(xembT_d[g * 128:(g + 1) * 128, :], stg[:])

        # ============ WEIGHTS (loop phase) ============
        w_pool = inner.enter_context(tc.tile_pool(name="w", bufs=1))
        W_comb = [w_pool.tile([128, 4 * H], WDT, name=f"wc{k}", tag=f"wc{k}")
                  for k in range(DC + HC)]
        for k in range(DC + HC):
            nc.sync.dma_start(W_comb[k][:], W_combT_d[k * 128:(k + 1) * 128, :])
        W_sT = [w_pool.tile([128, A], WDT, name=f"ws{k}", tag=f"ws{k}")
                for k in range(HC)]
        for k in range(HC):
            nc.sync.dma_start(W_sT[k][:], W_sT_d[k * 128:(k + 1) * 128, :])

        loop_sb = inner.enter_context(tc.tile_pool(name="lsb", bufs=2))
        loop_z = inner.enter_context(tc.tile_pool(name="lz", bufs=3))
        loop_q = inner.enter_context(tc.tile_pool(name="lq", bufs=2))
        enc_pool = inner.enter_context(tc.tile_pool(name="encp", bufs=1))
        loop_ps = inner.enter_context(tc.tile_pool(name="lps", bufs=1, space="PSUM"))
        loop_ps2 = inner.enter_context(tc.tile_pool(name="lps2", bufs=1,
                                                    space="PSUM"))

        # enc resident in SBUF for the whole loop (identical every step)
        enc_all = []
        for j in range(BL):
            ea = enc_pool.tile([128, TC[j] * 512], BF16, name=f"ea{j}",
                               tag=f"ea{j}")
            nc.sync.dma_start(
                ea[:], enc_td[j][:, :].rearrange("(c p) d -> p c d", p=128))
            enc_all.append(ea)

        # prologue: W_hh part of step 0 with h=0 (initializes psum groups)
        pe_touch(W_comb[DC][:, 0:128])
        pe_touch(h_q[:, 0:HC * BL])
        for g in range(GC):
            for k in range(HC):
                nc.tensor.matmul(Ghh[:, g * BL:(g + 1) * BL],
                                 W_comb[DC + k][:, g * 128:(g + 1) * 128],
                                 h_q[:, k * BL:(k + 1) * BL],
                                 start=(k == 0), stop=(k == HC - 1),
                                 skip_group_check=True)
        # qd init (accum_bf is zeroed)
        for j in range(BL):
            nc.sync.dma_start(qd[0:1, j * 1024:(j + 1) * 1024],
                              accum_bf[32 * j:32 * j + 1, :])

        # ============ STEP LOOP ============
        _nsteps = int(os.environ.get("KBSTEPS", S))
        UNROLL = int(os.environ.get("KBUNROLL", 4))
        # Persistent energy PSUM tile, zeroed once: columns beyond Tp[j] are
        # never matmul-written, so they must hold 0 (exp(0)*mask0 = 0), not
        # garbage (exp(NaN/huge) would poison the softmax row sums).
        Eps = loop_ps.tile([128, 1024], F32, name="Eps", tag="Eps")
        nc.vector.memset(Eps[:], 0.0)

        def step_body(t4):
            # ---- head DMAs ----
            xet = loop_sb.tile([128, GC * BL], BF16, name="xet", tag="xet")
            src = xembT_d[:, bass.ds(t4, BL)].rearrange("(c p) b -> p c b", p=128)
            nc.sync.dma_start(xet[:], src)
            qbc = loop_q.tile([128, BL * 1024], BF16, name="qbc", tag="qbc")
            nc.sync.dma_start(qbc[:], qd[0:1, :].partition_broadcast(128))

            # ---- gates: ctx part ----
            for g in range(GC):
                for k in range(DC):
                    nc.tensor.matmul(G[:, g * BL:(g + 1) * BL],
                                     W_comb[k][:, g * 128:(g + 1) * 128],
                                     ctx_q[:, k * BL:(k + 1) * BL],
                                     start=(k == 0), stop=(k == DC - 1),
                                     skip_group_check=True)

            gates_f = loop_sb.tile([128, GC * BL], F32, name="gates_f", tag="gf")
            nc.vector.tensor_tensor(gates_f[:], G[:], xet[:], ALU.add)
            nc.vector.tensor_tensor(gates_f[:], gates_f[:], Ghh[:], ALU.add)

            # ifo chunks 0..23 -> cols 0:96 ; g chunks 24..31 -> cols 96:128
            t_ifo = loop_sb.tile([128, 3 * HC * BL], F32, name="t_ifo", tag="ti")
            nc.scalar.activation(t_ifo[:], gates_f[:, 0:3 * HC * BL], AF.Tanh,
                                 scale=0.5 / SG)
            t_g = loop_sb.tile([128, HC * BL], F32, name="t_g", tag="tg")
            nc.scalar.activation(t_g[:], gates_f[:, 3 * HC * BL:], AF.Tanh,
                                 scale=1.0 / SG)

            HB = HC * BL
            sl_i, sl_f, sl_o = (t_ifo[:, 0:HB], t_ifo[:, HB:2 * HB],
                                t_ifo[:, 2 * HB:3 * HB])
            sig_i = loop_sb.tile([128, HB], F32, name="sig_i", tag="si")
            nc.vector.tensor_scalar(sig_i[:], sl_i, 0.5, 0.5, ALU.mult, ALU.add)
            sig_f = loop_sb.tile([128, HB], F32, name="sig_f", tag="sf")
            nc.vector.tensor_scalar(sig_f[:], sl_f, 0.5, 0.5, ALU.mult, ALU.add)
            ig = loop_sb.tile([128, HB], F32, name="ig", tag="ig")
            nc.vector.tensor_tensor(ig[:], sig_i[:], t_g[:], ALU.mult)
            fc = loop_sb.tile([128, HB], F32, name="fc", tag="fcx")
            nc.vector.tensor_tensor(fc[:], sig_f[:], c_st[:], ALU.mult)
            cn = loop_sb.tile([128, HB], F32, name="cn", tag="cn")
            nc.vector.tensor_tensor(cn[:], fc[:], ig[:], ALU.add)
            tcn = loop_sb.tile([128, HB], F32, name="tcn", tag="tcn")
            nc.scalar.activation(tcn[:], cn[:], AF.Tanh)
            c05 = loop_sb.tile([128, HB], F32, name="c05", tag="c05")
            nc.vector.tensor_scalar(c05[:], c_st[:], ZC, None, ALU.mult)
            nc.vector.scalar_tensor_tensor(c_st[:], cn[:], 1.0 - ZC, c05[:],
                                           ALU.mult, ALU.add)
            sig_o = loop_sb.tile([128, HB], F32, name="sig_o", tag="so")
            nc.vector.tensor_scalar(sig_o[:], sl_o, 0.5, 0.5, ALU.mult, ALU.add)
            ot = loop_sb.tile([128, HB], F32, name="ot", tag="ot")
            nc.vector.tensor_tensor(ot[:], sig_o[:], tcn[:], ALU.mult)
            h05 = loop_sb.tile([128, HB], F32, name="h05", tag="h05")
            nc.vector.tensor_scalar(h05[:], h_bf[:], ZH, None, ALU.mult)
            nc.vector.scalar_tensor_tensor(h_bf[:], ot[:], 1.0 - ZH, h05[:],
                                           ALU.mult, ALU.add)
            nc.vector.tensor_scalar(h_q[:], h_bf[:], SX, None, ALU.mult)

            dst = hstk_d[:, bass.ds(t4, BL)].rearrange("(c p) b -> p c b", p=128)
            nc.sync.dma_start(dst, h_bf[:])

            # ---- s_t ----
            s_ps = loop_ps.tile([128, AC * BL], F32, name="s_ps", tag="s_ps")
            for a in range(AC):
                for k in range(HC):
                    nc.tensor.matmul(s_ps[:, a * BL:(a + 1) * BL],
                                     W_sT[k][:, a * 128:(a + 1) * 128],
                                     h_q[:, k * BL:(k + 1) * BL],
                                     start=(k == 0), stop=(k == HC - 1),
                                     skip_group_check=True)
            nc.vector.tensor_scalar(s_sb[:], s_ps[:], 1.0 / SG, None, ALU.mult)

            # ---- W_hh part of NEXT step's gates ----
            for g in range(GC):
                for k in range(HC):
                    nc.tensor.matmul(Ghh[:, g * BL:(g + 1) * BL],
                                     W_comb[DC + k][:, g * 128:(g + 1) * 128],
                                     h_q[:, k * BL:(k + 1) * BL],
                                     start=(k == 0), stop=(k == HC - 1),
                                     skip_group_check=True)

            # ---- attention ----
            for j in range(BL):
                for a in range(AC):
                    ai = a * BL + j
                    # e_t holds e/wfb (host pre-divided); z = q + e/wfb,
                    # tanh applies scale=wfb, bias=s.
                    z = loop_z.tile([128, 1024], BF16, name="z", tag="z")
                    nc.vector.tensor_tensor(
                        z[:, 0:Tp[j]], qbc[:, j * 1024:j * 1024 + Tp[j]],
                        e_t[j][a][:], ALU.add)
                    tv = loop_z.tile([128, 1024], BF16, name="tv", tag="tv")
                    nc.scalar.activation(tv[:, 0:Tp[j]], z[:, 0:Tp[j]], AF.Tanh,
                                         bias=s_sb[:, ai:ai + 1],
                                         scale=wfb_colf[:, a:a + 1])
                    kw = dict(start=(a == 0), stop=(a == AC - 1),
                              skip_group_check=True)
                    if j:
                        kw["tile_position"] = (0, 32 * j)
                    nc.tensor.matmul(Eps[32 * j:32 * j + 1, 0:512],
                                     vT_col[:, a:a + 1], tv[:, 0:512], **kw)
                    if Tp[j] > 512:
                        nc.tensor.matmul(Eps[32 * j:32 * j + 1, 512:Tp[j]],
                                         vT_col[:, a:a + 1], tv[:, 512:Tp[j]],
                                         **kw)

            # ---- softmax (rows {0,32,64,96}) ----
            En = loop_sb.tile([128, 1024], BF16, name="En", tag="En")
            nc.scalar.activation(En[:], Eps[:], AF.Exp)
            w1 = loop_sb.tile([128, 1024], BF16, name="w1", tag="w1")
            nc.vector.tensor_tensor(w1[:], En[:], mask01[:], ALU.mult)
            se = loop_sb.tile([128, 1], F32, name="se", tag="se")
            nc.vector.tensor_reduce(se[:], w1[:], mybir.AxisListType.X, ALU.add)
            rse = loop_sb.tile([128, 1], F32, name="rse", tag="rse")
            nc.vector.reciprocal(rse[:], se[:])
            nc.vector.tensor_scalar(w_att[:], w1[:], rse[:], None, ALU.mult)

            # ---- accum += w*finv (bf16) ; qd bounce ----
            wf = loop_sb.tile([128, 1024], BF16, name="wf", tag="wf")
            nc.vector.tensor_tensor(wf[:], w_att[:], finv[:], ALU.mult)
            nc.vector.tensor_tensor(accum_bf[:], accum_bf[:], wf[:], ALU.add)
            for j in range(BL):
                nc.sync.dma_start(qd[0:1, j * 1024:(j + 1) * 1024],
                                  accum_bf[32 * j:32 * j + 1, :])

            # ---- transpose w_att -> wts [128,(tc,b)] ----
            for t in range(TCmax):
                wtp = loop_ps2.tile([128, 128], BF16, name="wtp", tag="wtp")
                nc.tensor.transpose(wtp[:, :], w_att[:, t * 128:(t + 1) * 128],
                                    ident[:, :])
                nc.vector.tensor_copy(wts[:, t * BL:(t + 1) * BL],
                                      wtp[:, 0:128:32])

            # ---- ctx matvecs (enc-stationary) ----
            c_ps = loop_ps.tile([128, DC * BL], F32, name="c_ps", tag="c_ps")
            for j in range(BL):
                for dk in range(DC):
                    for t in range(TC[j]):
                        nc.tensor.matmul(
                            c_ps[:, dk * BL + j:dk * BL + j + 1],
                            enc_all[j][:, t * 512 + dk * 128:t * 512 + (dk + 1) * 128],
                            wts[:, t * BL + j:t * BL + j + 1],
                            start=(t == 0), stop=(t == TC[j] - 1),
                            skip_group_check=True)
            nc.vector.tensor_copy(ctxT_sb[:], c_ps[:])
            nc.vector.tensor_scalar(ctx_q[:], c_ps[:], SX, None, ALU.mult)
            dst = cstk_d[:, bass.ds(t4, BL)].rearrange("(c p) b -> p c b", p=128)
            nc.sync.dma_start(dst, ctxT_sb[:])

        with tc.For_i(0, _nsteps * BL, UNROLL * BL,
                      hint_engines=(ET.PE, ET.Activation, ET.DVE, ET.SP)) as t4:
            for s in range(UNROLL):
                step_body(t4 + s * BL)

        # ============ READOUT ============
        inner.close()
        post_sb = ctx.enter_context(tc.tile_pool(name="post_sb", bufs=1))
        post_st = ctx.enter_context(tc.tile_pool(name="post_st", bufs=2))
        post_ps = ctx.enter_context(tc.tile_pool(name="post_ps", bufs=2,
                                                 space="PSUM"))

        xro = []
        for k in range(HC):
            tl = post_sb.tile([128, NS], BF16, name=f"xh{k}", tag=f"xh{k}")
            nc.sync.dma_start(tl[:], hstk_d[k * 128:(k + 1) * 128, :])
            xro.append(tl)
        for k in range(EC):
            tl = post_sb.tile([128, NS], BF16, name=f"xe{k}", tag=f"xe{k}")
            nc.sync.dma_start(tl[:], embT_d[k * 128:(k + 1) * 128, :])
            xro.append(tl)
        for k in range(DC):
            tl = post_sb.tile([128, NS], BF16, name=f"xc{k}", tag=f"xc{k}")
            nc.sync.dma_start(tl[:], cstk_d[k * 128:(k + 1) * 128, :])
            xro.append(tl)
        W_roe = [post_sb.tile([128, RO // 2], BF16, name=f"wre{k}", tag=f"wre{k}")
                 for k in range(XROC)]
        W_roo = [post_sb.tile([128, RO // 2], BF16, name=f"wro{k}", tag=f"wro{k}")
                 for k in range(XROC)]
        for k in range(XROC):
            nc.sync.dma_start(W_roe[k][:], W_roT_e_d[k * 128:(k + 1) * 128, :])
            nc.sync.dma_start(W_roo[k][:], W_roT_o_d[k * 128:(k + 1) * 128, :])
        b_ro_e = post_sb.tile([128, ROC], F32)
        nc.sync.dma_start(b_ro_e[:], b_ro_e_d[:, :])
        b_ro_o = post_sb.tile([128, ROC], F32)
        nc.sync.dma_start(b_ro_o[:], b_ro_o_d[:, :])
        b_out_col = post_sb.tile([128, VC], F32)
        nc.sync.dma_start(b_out_col[:], b_out_d[:, :])
        pe_touch(xro[0][:, 0:128])
        pe_touch(W_roe[0][:, 0:128])
        pe_touch(W_roo[0][:, 0:128])

        maxo = []
        for oc in range(ROC):
            Re = post_ps.tile([128, NS], F32, name="Re", tag="Re")
            for k in range(XROC):
                nc.tensor.matmul(Re[:], W_roe[k][:, oc * 128:(oc + 1) * 128],
                                 xro[k][:], start=(k == 0), stop=(k == XROC - 1))
            t1 = post_st.tile([128, NS], F32, name="t1", tag="t1")
            nc.scalar.activation(t1[:], Re[:], AF.Identity,
                                 bias=b_ro_e[:, oc:oc + 1], scale=1.0)
            Ro = post_ps.tile([128, NS], F32, name="Ro", tag="Re")
            for k in range(XROC):
                nc.tensor.matmul(Ro[:], W_roo[k][:, oc * 128:(oc + 1) * 128],
                                 xro[k][:], start=(k == 0), stop=(k == XROC - 1))
            t2 = post_st.tile([128, NS], F32, name="t2", tag="t2")
            nc.scalar.activation(t2[:], Ro[:], AF.Identity,
                                 bias=b_ro_o[:, oc:oc + 1], scale=1.0)
            mo = post_sb.tile([128, NS], BF16, name=f"mo{oc}", tag=f"mo{oc}")
            nc.vector.tensor_tensor(mo[:], t1[:], t2[:], ALU.max)
            maxo.append(mo)

        wo_pool = ctx.enter_context(tc.tile_pool(name="wo", bufs=6))
        first = True
        for vc in range(VC):
            wo = [wo_pool.tile([128, 128], BF16, name=f"wo{vc}_{k}", tag=f"wok{k}")
                  for k in range(ROC)]
            for k in range(ROC):
                nc.sync.dma_start(wo[k][:],
                                  W_outT_d[k * 128:(k + 1) * 128,
                                           vc * 128:(vc + 1) * 128])
            if first:
                pe_touch(wo[0][:, 0:128])
                pe_touch(maxo[0][:, 0:128])
                first = False
            L = post_ps.tile([128, NS], F32, name="L", tag="L")
            for k in range(ROC):
                nc.tensor.matmul(L[:], wo[k][:], maxo[k][:],
                                 start=(k == 0), stop=(k == ROC - 1))
            lo = post_st.tile([128, NS], F32, name="lo", tag="lo")
            nc.scalar.activation(lo[:], L[:], AF.Identity,
                                 bias=b_out_col[:, vc:vc + 1], scale=1.0)
            nc.sync.dma_start(out_d[vc * 128:(vc + 1) * 128, :], lo[:])

    return nc


def check_waits(nc, cap_note=""):
    bad = []
    for fn in nc.m.functions:
        for bb in fn.blocks:
            for inst in bb.instructions:
                c = inst.concise()
                nw = c.count("wait:")
                eng = c.split()[0] if c.split() else "?"
                if nw >= 2 and eng in ("PE", "ACT", "DVE", "PL"):
                    bad.append((nw, c[:180]))
    for nw, c in bad:
        print("WAITS", nw, c)
    return bad


def _prep_core(inputs, order, Tp, core):
    enc = np.asarray(inputs["encoder_outputs"], np.float32)
    labels = np.asarray(inputs["labels"])
    lens = np.asarray(inputs["enc_seq_len"], np.int64)
    embed = np.asarray(inputs["embed"], np.float32)

    bidx = [int(order[j * NCORE + core]) for j in range(BL)]
    m = {}
    for j in range(BL):
        b = bidx[j]
        ep = np.zeros((Tp[j], D), np.float32)
        ep[:T] = enc[b, :Tp[j] if Tp[j] <= T else T]
        m[f"enc_td{j}"] = _bf(ep)
        m[f"encT{j}"] = _bf(ep.T)
    emb = np.zeros((BL, S, E), np.float32)
    for j in range(BL):
        b = bidx[j]
        emb[j, 1:] = embed[labels[b, :S - 1].astype(np.int64)]
    embT = emb.transpose(2, 1, 0).reshape(E, NS)
    m["embT"] = _bf(embT)
    mask01 = np.zeros((BL, 1024), np.float32)
    for j in range(BL):
        mask01[j, :int(lens[bidx[j]])] = 1.0
    m["mask01"] = _bf(mask01)
    return m, bidx


def kernel(**inputs):
    lens = np.asarray(inputs["enc_seq_len"], np.int64)
    order = np.argsort(-lens, kind="stable")
    Tp = []
    for j in range(BL):
        mx = max(int(lens[order[j * NCORE + i]]) for i in range(NCORE))
        Tp.append(min(1024, ((mx + 127) // 128) * 128))

    perm = _gate_perm()
    W_ih = np.asarray(inputs["W_ih"], np.float32)[perm]
    W_hh = np.asarray(inputs["W_hh"], np.float32)[perm]
    b_sum = (np.asarray(inputs["b_ih"], np.float32)
             + np.asarray(inputs["b_hh"], np.float32))[perm]
    wfb = np.asarray(inputs["W_fb"], np.float32)[:, 0]
    wfb_safe = np.where(wfb >= 0, np.maximum(wfb, 1e-3),
                        np.minimum(wfb, -1e-3))
    shared = {
        "W_combT": _w8(np.concatenate([W_ih[:, E:].T, W_hh.T], 0)),
        "W_ih_embT": _bf(W_ih[:, :E].T),
        "W_encT": _bf(np.asarray(inputs["W_enc"], np.float32).T
                      / wfb_safe[None, :]),
        "W_sT": _w8(np.asarray(inputs["W_s"], np.float32).T),
        "wfert_col": _bf(np.asarray(inputs["W_fert"],
                                    np.float32).reshape(DC, 128).T),
        "vT_col": _bf(np.asarray(inputs["v_att"], np.float32).reshape(AC, 128).T),
        "wfb_colf": np.ascontiguousarray(
            wfb_safe.reshape(AC, 128).T.astype(np.float32)),
        "b_enc_col": np.ascontiguousarray(
            (np.asarray(inputs["b_enc"], np.float32) / wfb_safe)
            .reshape(AC, 128).T),
        "b_comb": np.ascontiguousarray(
            (b_sum * SG).reshape(GC, 128).T),
        "W_roT_e": _bf(np.asarray(inputs["W_ro"], np.float32)[0::2].T),
        "W_roT_o": _bf(np.asarray(inputs["W_ro"], np.float32)[1::2].T),
        "b_ro_e": np.ascontiguousarray(
            np.asarray(inputs["b_ro"], np.float32)[0::2].reshape(ROC, 128).T),
        "b_ro_o": np.ascontiguousarray(
            np.asarray(inputs["b_ro"], np.float32)[1::2].reshape(ROC, 128).T),
        "W_outT": _bf(np.asarray(inputs["W_out"], np.float32).T),
        "b_out_col": np.ascontiguousarray(
            np.asarray(inputs["b_out"], np.float32).reshape(VC, 128).T),
    }

    in_maps = []
    bidx_all = []
    for c in range(NCORE):
        m, bidx = _prep_core(inputs, order, Tp, c)
        m.update(shared)
        in_maps.append(m)
        bidx_all.append(bidx)

    nc = build_nc(Tp)
    nc.finalize()
    from concourse.bass_utils import run_bass_kernel_spmd
    trace = bool(os.environ.get("BASS_KERNEL_TRACE"))
    res = run_bass_kernel_spmd(nc, in_maps, core_ids=list(range(NCORE)),
                               trace=trace)
    global LAST_EXEC_NS, LAST_OUTS, LAST_META
    LAST_EXEC_NS = res.exec_time_ns
    outs = res.results
    LAST_OUTS = outs
    LAST_META = (order, Tp, bidx_all)

    logits = np.zeros((B, S, V), np.float32)
    for c in range(NCORE):
        o = outs[c]["out"].reshape(V, S, BL)
        for j in range(BL):
            logits[bidx_all[c][j]] = o[:, :, j].T
    return logits


if __name__ == "__main__":
    nc = build_nc([1024, 896, 768, 640])
    bad = check_waits(nc)
    print(f"{len(bad)} instructions with >=2 waits")


# revision 49
# speedup vs baseline: 1.2140x; 1.0227x over previous
"""Attention-LSTM decoder (B=32, T=1000, S=100, D=512, A=1024, H=1024,
E=640, V=10240, P=1024) on 8 trn2 NeuronCores.

Sharding: data-parallel over batch, 4 batches per core (one per "slot").
Batches are sorted by enc_seq_len; slot j holds ranks [j*8:(j+1)*8] so the
padded time extent Tp[j] (multiple of 128) is shared by all 8 cores and the
SPMD graph is identical across cores.

v2 design (vs. baseline):
  - gate weights W_comb / W_s stored fp8e4m3 (x64 scale), moving operands
    h / ctx quantized to fp8 (x16); descale folded into activation scales.
  - LSTM sigmoids computed as 0.5*tanh(0.5x)+0.5 so the whole kernel uses
    one ACT table (exp_and_others: tanh+exp+identity) -- zero table loads
    inside the step loop.
  - attention z-pass = one STT per (slot, a-chunk) over the full Tp extent,
    alternating DVE / Pool engines; tanh on ACT with bias=s (per-partition).
  - energies accumulate into one PSUM tile [128,1024] at partition rows
    {0,32,64,96}; single Exp over the tile, multiplicative {0,1} mask,
    single reduce/recip/scale -- softmax is 1 ACT + 4 DVE ops total.
  - accum/finv/mask/w_att all live in the stride-32 row layout.
  - enc streamed per step as 4 large DMAs ([128, TC*512] per slot);
    q broadcast as one [128, 4096] DMA from a [1,4096] DRAM bounce.
  - ctx matvecs enc-stationary as before; w_att transposed via full
    128x128 PE transposes + stride-32 free-dim gather copies.
"""
import sys

sys.path.insert(0, "/opt/trn_rl_repo")

import os
import numpy as np
import ml_dtypes
from contextlib import ExitStack

import concourse.bass as bass
import concourse.tile as tile
import concourse.mybir as mybir
from concourse import bacc
from concourse.masks import make_identity

DT = mybir.dt
F32 = DT.float32
BF16 = DT.bfloat16
FP8 = DT.float8e4
AF = mybir.ActivationFunctionType
ALU = mybir.AluOpType
ET = mybir.EngineType

B, T, S = 32, 1000, 100
D, A, H, E, V, RO = 512, 1024, 1024, 640, 10240, 1024
ZH, ZC = 0.05, 0.15
NCORE = 8
BL = B // NCORE          # 4 batches (slots) per core
NS = S * BL              # 400 step-batch columns
GC = 4 * H // 128        # 32 gate chunks
HC = H // 128            # 8
AC = A // 128            # 8
DC = D // 128            # 4
EC = E // 128            # 5
ROC = RO // 2 // 128     # 4 chunks per maxout half
VC = V // 128            # 80 vocab chunks
XROC = (H + E + D) // 128  # 17 readout K-chunks

USE_FP8 = os.environ.get("KBFP8", "1") != "0"
SW = 64.0 if USE_FP8 else 1.0    # weight scale
SX = 16.0 if USE_FP8 else 1.0    # moving (h/ctx) scale
SG = SW * SX                      # psum scale for gates/s
WDT = FP8 if USE_FP8 else BF16

bf16 = ml_dtypes.bfloat16
f8 = ml_dtypes.float8_e4m3
LAST_EXEC_NS = None
LAST_OUTS = None
LAST_META = None


def _bf(a):
    return np.ascontiguousarray(np.asarray(a, dtype=np.float32)).astype(bf16)


def _w8(a):
    a = np.asarray(a, dtype=np.float32) * SW
    return np.ascontiguousarray(a).astype(f8 if USE_FP8 else bf16)


# gate-permutation: reference gate order is [i|f|g|o]; we reorder rows to
# [i|f|o|g] so the three sigmoids are contiguous.
def _gate_perm():
    idx = np.arange(4 * H)
    return np.concatenate([idx[0:2 * H], idx[3 * H:4 * H], idx[2 * H:3 * H]])


def build_nc(Tp, debug=False):
    TC = [t // 128 for t in Tp]
    TCmax = max(TC)
    nc = bacc.Bacc("TRN2", target_bir_lowering=False)

    def param(name, shape, dt=BF16):
        return nc.declare_dram_parameter(name, list(shape), dt, isOutput=False)

    enc_td = [param(f"enc_td{j}", [Tp[j], D]) for j in range(BL)]
    encT = [param(f"encT{j}", [D, Tp[j]]) for j in range(BL)]
    embT_d = param("embT", [E, NS])
    W_combT_d = param("W_combT", [D + H, 4 * H], WDT)
    W_ih_embT_d = param("W_ih_embT", [E, 4 * H])
    W_encT_d = param("W_encT", [D, A])
    W_sT_d = param("W_sT", [H, A], WDT)
    wfert_col_d = param("wfert_col", [128, DC])
    vT_col_d = param("vT_col", [128, AC])
    wfb_colf_d = param("wfb_colf", [128, AC], F32)
    b_enc_col_d = param("b_enc_col", [128, AC], F32)
    b_comb_d = param("b_comb", [128, GC], F32)     # pre-scaled by SG on host
    mask01_d = param("mask01", [BL, 1024])         # {0,1} rows
    W_roT_e_d = param("W_roT_e", [H + E + D, RO // 2])
    W_roT_o_d = param("W_roT_o", [H + E + D, RO // 2])
    b_ro_e_d = param("b_ro_e", [128, ROC], F32)
    b_ro_o_d = param("b_ro_o", [128, ROC], F32)
    W_outT_d = param("W_outT", [RO // 2, V])
    b_out_d = param("b_out_col", [128, VC], F32)
    out_d = nc.declare_dram_parameter("out", [V, NS], F32, isOutput=True)

    qd = nc.dram_tensor("qd", [1, BL * 1024], BF16)
    hstk_d = nc.dram_tensor("hstk", [H, NS], BF16)
    cstk_d = nc.dram_tensor("cstk", [D, NS], BF16)
    xembT_d = nc.dram_tensor("xembT", [4 * H, NS], BF16)

    with ExitStack() as ctx:
        tc = ctx.enter_context(tile.TileContext(nc))

        # ---------------- persistent pools ----------------
        persist = ctx.enter_context(tc.tile_pool(name="persist", bufs=1))
        ident = persist.tile([128, 128], BF16)
        make_identity(nc, ident[:])
        vT_col = persist.tile([128, AC], BF16)
        nc.sync.dma_start(vT_col[:], vT_col_d[:, :])
        wfb_colf = persist.tile([128, AC], F32)
        nc.sync.dma_start(wfb_colf[:], wfb_colf_d[:, :])
        wfert_col = persist.tile([128, DC], BF16)
        nc.sync.dma_start(wfert_col[:], wfert_col_d[:, :])
        b_enc_col = persist.tile([128, AC], F32)
        nc.sync.dma_start(b_enc_col[:], b_enc_col_d[:, :])
        b_comb = persist.tile([128, GC], F32)
        nc.sync.dma_start(b_comb[:], b_comb_d[:, :])
        mask01 = persist.tile([128, 1024], BF16)
        nc.vector.memset(mask01[:], 0.0)
        for j in range(BL):
            nc.sync.dma_start(mask01[32 * j:32 * j + 1, :], mask01_d[j:j + 1, :])

        h_bf = persist.tile([128, HC * BL], BF16)
        h_q = persist.tile([128, HC * BL], WDT)
        c_st = persist.tile([128, HC * BL], F32)
        ctxT_sb = persist.tile([128, DC * BL], BF16)
        ctx_q = persist.tile([128, DC * BL], WDT)
        s_sb = persist.tile([128, AC * BL], F32)
        accum_bf = persist.tile([128, 1024], BF16)  # rows {0,32,64,96}
        w_att = persist.tile([128, 1024], BF16)
        finv = persist.tile([128, 1024], BF16)     # rows {0,32,64,96}, x0.5
        wts = persist.tile([128, TCmax * BL], BF16)
        for t_ in (h_bf, h_q, c_st, ctxT_sb, ctx_q, s_sb, accum_bf,
                   w_att, finv, wts):
            nc.vector.memset(t_[:], 0.0)

        inner = ctx.enter_context(ExitStack())
        e_pool = inner.enter_context(tc.tile_pool(name="e", bufs=1))
        e_t = [[e_pool.tile([128, Tp[j]], BF16, name=f"e_{j}_{a}", tag=f"e{j}{a}")
                for a in range(AC)] for j in range(BL)]

        ppsum = ctx.enter_context(tc.tile_pool(name="ppsum", bufs=1, space="PSUM"))
        G = ppsum.tile([128, GC * BL], F32)
        Ghh = ppsum.tile([128, GC * BL], F32)
        trash_holder = []

        def pe_touch(ap):
            # phase-scoped trash tile (pre/post only; fp8 touches are no-ops)
            if ap.dtype not in (BF16,) or not trash_holder:
                return
            trash_ps = trash_holder[0]
            p = ap.shape[0]
            nc.tensor.transpose(trash_ps[0:min(ap.shape[1], 128), 0:p],
                                ap[:, 0:min(ap.shape[1], 128)], ident[0:p, 0:p])

        # ============ PRECOMPUTE PHASE ============
        with ExitStack() as pre:
            pre_sb = pre.enter_context(tc.tile_pool(name="pre_sb", bufs=1))
            pre_st = pre.enter_context(tc.tile_pool(name="pre_st", bufs=2))
            pre_ps = pre.enter_context(tc.tile_pool(name="pre_ps", bufs=1,
                                                    space="PSUM"))
            trash_holder.append(pre_ps.tile([128, 128], BF16, name="trash_pre"))

            W_encT = [pre_sb.tile([128, A], BF16, name=f"wenc{k}", tag=f"we{k}")
                      for k in range(DC)]
            for k in range(DC):
                nc.sync.dma_start(W_encT[k][:], W_encT_d[k * 128:(k + 1) * 128, :])
            pe_touch(W_encT[0][:, 0:128])

            for j in range(BL):
                ercs = [pre_st.tile([128, Tp[j]], BF16, name=f"erc{j}{k}",
                                    tag=f"erc{k}") for k in range(DC)]
                for k in range(DC):
                    nc.sync.dma_start(ercs[k][:], encT[j][k * 128:(k + 1) * 128, :])
                    pe_touch(ercs[k][:, 0:128])
                for a in range(AC):
                    pe2 = pre_ps.tile([128, 1024], F32, name="pe_e2", tag="pe_e2")
                    for k in range(DC):
                        for n0 in range(0, Tp[j], 512):
                            n1 = min(n0 + 512, Tp[j])
                            nc.tensor.matmul(pe2[:, n0:n1],
                                             W_encT[k][:, a * 128:(a + 1) * 128],
                                             ercs[k][:, n0:n1],
                                             start=(k == 0), stop=(k == DC - 1))
                    nc.scalar.activation(e_t[j][a][:], pe2[:, 0:Tp[j]],
                                         AF.Identity,
                                         bias=b_enc_col[:, a:a + 1], scale=1.0)
                pf = pre_ps.tile([1, 1024], F32, name="pf", tag="pf")
                for k in range(DC):
                    for n0 in range(0, Tp[j], 512):
                        n1 = min(n0 + 512, Tp[j])
                        nc.tensor.matmul(pf[0:1, n0:n1], wfert_col[:, k:k + 1],
                                         ercs[k][:, n0:n1],
                                         start=(k == 0), stop=(k == DC - 1))
                # finv = 0.5*sigmoid(x) = 0.25*tanh(0.5x) + 0.25  (no table sw)
                fstage = pre_st.tile([1, 1024], F32, name="fstage", tag="fstage")
                nc.scalar.activation(fstage[0:1, 0:Tp[j]], pf[0:1, 0:Tp[j]],
                                     AF.Tanh, scale=0.5)
                fst2 = pre_st.tile([1, 1024], BF16, name="fst2", tag="fst2")
                nc.vector.tensor_scalar(fst2[0:1, 0:Tp[j]], fstage[0:1, 0:Tp[j]],
                                        0.25, 0.25, ALU.mult, ALU.add)
                nc.sync.dma_start(finv[32 * j:32 * j + 1, 0:Tp[j]],
                                  fst2[0:1, 0:Tp[j]])

            embT_sb = [pre_sb.tile([128, NS], BF16, name=f"embs{k}", tag=f"em{k}")
                       for k in range(EC)]
            for k in range(EC):
                nc.sync.dma_start(embT_sb[k][:], embT_d[k * 128:(k + 1) * 128, :])
            W_ie = [pre_sb.tile([128, 4 * H], BF16, name=f"wie{k}", tag=f"wi{k}")
                    for k in range(EC)]
            for k in range(EC):
                nc.sync.dma_start(W_ie[k][:], W_ih_embT_d[k * 128:(k + 1) * 128, :])
            pe_touch(W_ie[0][:, 0:128])
            pe_touch(embT_sb[0][:, 0:128])
            for g in range(GC):
                px = pre_ps.tile([128, NS], F32, name="px", tag="pe_e2")
                for k in range(EC):
                    nc.tensor.matmul(px[:], W_ie[k][:, g * 128:(g + 1) * 128],
                                     embT_sb[k][:], start=(k == 0),
                                     stop=(k == EC - 1))
                # xemb scaled by SG, bias pre-scaled on host
                stg = pre_st.tile([128, NS], BF16, name="xstg", tag="xstg")
                nc.scalar.activation(stg[:], px[:], AF.Identity,
                                     bias=b_comb[:, g:g + 1], scale=SG)
                nc.sync.dma_start(xembT_d[g * 128:(g + 1) * 128, :], stg[:])

        trash_holder.clear()

        # ============ WEIGHTS (loop phase) ============
        w_pool = inner.enter_context(tc.tile_pool(name="w", bufs=1))
        W_comb = [w_pool.tile([128, 4 * H], WDT, name=f"wc{k}", tag=f"wc{k}")
                  for k in range(DC + HC)]
        for k in range(DC + HC):
            nc.sync.dma_start(W_comb[k][:], W_combT_d[k * 128:(k + 1) * 128, :])
        W_sT = [w_pool.tile([128, A], WDT, name=f"ws{k}", tag=f"ws{k}")
                for k in range(HC)]
        for k in range(HC):
            nc.sync.dma_start(W_sT[k][:], W_sT_d[k * 128:(k + 1) * 128, :])

        loop_sb = inner.enter_context(tc.tile_pool(name="lsb", bufs=2))
        loop_z = inner.enter_context(tc.tile_pool(name="lz", bufs=3))
        loop_q = inner.enter_context(tc.tile_pool(name="lq", bufs=2))
        enc_pool = inner.enter_context(tc.tile_pool(name="encp", bufs=1))
        loop_ps = inner.enter_context(tc.tile_pool(name="lps", bufs=1, space="PSUM"))
        loop_ps2 = inner.enter_context(tc.tile_pool(name="lps2", bufs=2,
                                                    space="PSUM"))

        # enc resident in SBUF for the whole loop (identical every step)
        enc_all = []
        for j in range(BL):
            ea = enc_pool.tile([128, TC[j] * 512], BF16, name=f"ea{j}",
                               tag=f"ea{j}")
            nc.sync.dma_start(
                ea[:], enc_td[j][:, :].rearrange("(c p) d -> p c d", p=128))
            enc_all.append(ea)

        # prologue: W_hh part of step 0 with h=0 (initializes psum groups)
        pe_touch(W_comb[DC][:, 0:128])
        pe_touch(h_q[:, 0:HC * BL])
        for g in range(GC):
            for k in range(HC):
                nc.tensor.matmul(Ghh[:, g * BL:(g + 1) * BL],
                                 W_comb[DC + k][:, g * 128:(g + 1) * 128],
                                 h_q[:, k * BL:(k + 1) * BL],
                                 start=(k == 0), stop=(k == HC - 1),
                                 skip_group_check=True)
        # qd init (accum_bf is zeroed)
        for j in range(BL):
            nc.sync.dma_start(qd[0:1, j * 1024:(j + 1) * 1024],
                              accum_bf[32 * j:32 * j + 1, :])

        # ============ STEP LOOP ============
        _nsteps = int(os.environ.get("KBSTEPS", S))
        UNROLL = int(os.environ.get("KBUNROLL", 4))
        # Persistent energy PSUM tile, zeroed once: columns beyond Tp[j] are
        # never matmul-written, so they must hold 0 (exp(0)*mask0 = 0), not
        # garbage (exp(NaN/huge) would poison the softmax row sums).
        Eps = loop_ps.tile([128, 1024], F32, name="Eps", tag="Eps")
        nc.vector.memset(Eps[:], 0.0)

        def step_body(t4):
            # ---- head DMAs ----
            xet = loop_sb.tile([128, GC * BL], BF16, name="xet", tag="xet")
            src = xembT_d[:, bass.ds(t4, BL)].rearrange("(c p) b -> p c b", p=128)
            nc.sync.dma_start(xet[:], src)
            qbc = loop_q.tile([128, BL * 1024], BF16, name="qbc", tag="qbc")
            nc.sync.dma_start(qbc[:], qd[0:1, :].partition_broadcast(128))

            # ---- gates: ctx part ----
            for g in range(GC):
                for k in range(DC):
                    nc.tensor.matmul(G[:, g * BL:(g + 1) * BL],
                                     W_comb[k][:, g * 128:(g + 1) * 128],
                                     ctx_q[:, k * BL:(k + 1) * BL],
                                     start=(k == 0), stop=(k == DC - 1),
                                     skip_group_check=True)

            gates_f = loop_sb.tile([128, GC * BL], F32, name="gates_f", tag="gf")
            nc.vector.tensor_tensor(gates_f[:], G[:], xet[:], ALU.add)
            nc.vector.tensor_tensor(gates_f[:], gates_f[:], Ghh[:], ALU.add)

            # ifo chunks 0..23 -> cols 0:96 ; g chunks 24..31 -> cols 96:128
            t_ifo = loop_sb.tile([128, 3 * HC * BL], F32, name="t_ifo", tag="ti")
            nc.scalar.activation(t_ifo[:], gates_f[:, 0:3 * HC * BL], AF.Tanh,
                                 scale=0.5 / SG)
            t_g = loop_sb.tile([128, HC * BL], F32, name="t_g", tag="tg")
            nc.scalar.activation(t_g[:], gates_f[:, 3 * HC * BL:], AF.Tanh,
                                 scale=1.0 / SG)

            HB = HC * BL
            sl_i, sl_f, sl_o = (t_ifo[:, 0:HB], t_ifo[:, HB:2 * HB],
                                t_ifo[:, 2 * HB:3 * HB])
            sig_i = loop_sb.tile([128, HB], F32, name="sig_i", tag="si")
            nc.vector.tensor_scalar(sig_i[:], sl_i, 0.5, 0.5, ALU.mult, ALU.add)
            sig_f = loop_sb.tile([128, HB], F32, name="sig_f", tag="sf")
            nc.vector.tensor_scalar(sig_f[:], sl_f, 0.5, 0.5, ALU.mult, ALU.add)
            ig = loop_sb.tile([128, HB], F32, name="ig", tag="ig")
            nc.vector.tensor_tensor(ig[:], sig_i[:], t_g[:], ALU.mult)
            fc = loop_sb.tile([128, HB], F32, name="fc", tag="fcx")
            nc.vector.tensor_tensor(fc[:], sig_f[:], c_st[:], ALU.mult)
            cn = loop_sb.tile([128, HB], F32, name="cn", tag="cn")
            nc.vector.tensor_tensor(cn[:], fc[:], ig[:], ALU.add)
            tcn = loop_sb.tile([128, HB], F32, name="tcn", tag="tcn")
            nc.scalar.activation(tcn[:], cn[:], AF.Tanh)
            c05 = loop_sb.tile([128, HB], F32, name="c05", tag="c05")
            nc.vector.tensor_scalar(c05[:], c_st[:], ZC, None, ALU.mult)
            nc.vector.scalar_tensor_tensor(c_st[:], cn[:], 1.0 - ZC, c05[:],
                                           ALU.mult, ALU.add)
            sig_o = loop_sb.tile([128, HB], F32, name="sig_o", tag="so")
            nc.vector.tensor_scalar(sig_o[:], sl_o, 0.5, 0.5, ALU.mult, ALU.add)
            ot = loop_sb.tile([128, HB], F32, name="ot", tag="ot")
            nc.vector.tensor_tensor(ot[:], sig_o[:], tcn[:], ALU.mult)
            h05 = loop_sb.tile([128, HB], F32, name="h05", tag="h05")
            nc.vector.tensor_scalar(h05[:], h_bf[:], ZH, None, ALU.mult)
            nc.vector.scalar_tensor_tensor(h_bf[:], ot[:], 1.0 - ZH, h05[:],
                                           ALU.mult, ALU.add)
            nc.vector.tensor_scalar(h_q[:], h_bf[:], SX, None, ALU.mult)

            dst = hstk_d[:, bass.ds(t4, BL)].rearrange("(c p) b -> p c b", p=128)
            nc.sync.dma_start(dst, h_bf[:])

            # ---- s_t ----
            s_ps = loop_ps.tile([128, AC * BL], F32, name="s_ps", tag="s_ps")
            for a in range(AC):
                for k in range(HC):
                    nc.tensor.matmul(s_ps[:, a * BL:(a + 1) * BL],
                                     W_sT[k][:, a * 128:(a + 1) * 128],
                                     h_q[:, k * BL:(k + 1) * BL],
                                     start=(k == 0), stop=(k == HC - 1),
                                     skip_group_check=True)
            nc.vector.tensor_scalar(s_sb[:], s_ps[:], 1.0 / SG, None, ALU.mult)

            # ---- W_hh part of NEXT step's gates ----
            for g in range(GC):
                for k in range(HC):
                    nc.tensor.matmul(Ghh[:, g * BL:(g + 1) * BL],
                                     W_comb[DC + k][:, g * 128:(g + 1) * 128],
                                     h_q[:, k * BL:(k + 1) * BL],
                                     start=(k == 0), stop=(k == HC - 1),
                                     skip_group_check=True)

            # ---- attention ----
            for j in range(BL):
                for a in range(AC):
                    ai = a * BL + j
                    # e_t holds e/wfb (host pre-divided); z = q + e/wfb,
                    # tanh applies scale=wfb, bias=s.
                    z = loop_z.tile([128, 1024], BF16, name="z", tag="z")
                    nc.vector.tensor_tensor(
                        z[:, 0:Tp[j]], qbc[:, j * 1024:j * 1024 + Tp[j]],
                        e_t[j][a][:], ALU.add)
                    tv = loop_z.tile([128, 1024], BF16, name="tv", tag="tv")
                    nc.scalar.activation(tv[:, 0:Tp[j]], z[:, 0:Tp[j]], AF.Tanh,
                                         bias=s_sb[:, ai:ai + 1],
                                         scale=wfb_colf[:, a:a + 1])
                    kw = dict(start=(a == 0), stop=(a == AC - 1),
                              skip_group_check=True)
                    if j:
                        kw["tile_position"] = (0, 32 * j)
                    nc.tensor.matmul(Eps[32 * j:32 * j + 1, 0:512],
                                     vT_col[:, a:a + 1], tv[:, 0:512], **kw)
                    if Tp[j] > 512:
                        nc.tensor.matmul(Eps[32 * j:32 * j + 1, 512:Tp[j]],
                                         vT_col[:, a:a + 1], tv[:, 512:Tp[j]],
                                         **kw)

            # ---- softmax (rows {0,32,64,96}) ----
            En = loop_sb.tile([128, 1024], BF16, name="En", tag="En")
            nc.scalar.activation(En[:], Eps[:], AF.Exp)
            w1 = loop_sb.tile([128, 1024], BF16, name="w1", tag="w1")
            nc.vector.tensor_tensor(w1[:], En[:], mask01[:], ALU.mult)
            se = loop_sb.tile([128, 1], F32, name="se", tag="se")
            nc.vector.tensor_reduce(se[:], w1[:], mybir.AxisListType.X, ALU.add)
            rse = loop_sb.tile([128, 1], F32, name="rse", tag="rse")
            nc.vector.reciprocal(rse[:], se[:])
            nc.vector.tensor_scalar(w_att[:], w1[:], rse[:], None, ALU.mult)

            # ---- accum += w*finv (bf16) ; qd bounce ----
            wf = loop_sb.tile([128, 1024], BF16, name="wf", tag="wf")
            nc.vector.tensor_tensor(wf[:], w_att[:], finv[:], ALU.mult)
            nc.vector.tensor_tensor(accum_bf[:], accum_bf[:], wf[:], ALU.add)
            for j in range(BL):
                nc.sync.dma_start(qd[0:1, j * 1024:(j + 1) * 1024],
                                  accum_bf[32 * j:32 * j + 1, :])

            # ---- transpose w_att -> wts [128,(tc,b)] ----
            for t in range(TCmax):
                wtp = loop_ps2.tile([128, 128], BF16, name="wtp", tag="wtp")
                nc.tensor.transpose(wtp[:, :], w_att[:, t * 128:(t + 1) * 128],
                                    ident[:, :])
                nc.vector.tensor_copy(wts[:, t * BL:(t + 1) * BL],
                                      wtp[:, 0:128:32])

            # ---- ctx matvecs (enc-stationary) ----
            c_ps = loop_ps.tile([128, DC * BL], F32, name="c_ps", tag="c_ps")
            for j in range(BL):
                for dk in range(DC):
                    for t in range(TC[j]):
                        nc.tensor.matmul(
                            c_ps[:, dk * BL + j:dk * BL + j + 1],
                            enc_all[j][:, t * 512 + dk * 128:t * 512 + (dk + 1) * 128],
                            wts[:, t * BL + j:t * BL + j + 1],
                            start=(t == 0), stop=(t == TC[j] - 1),
                            skip_group_check=True)
            nc.vector.tensor_copy(ctxT_sb[:], c_ps[:])
            nc.vector.tensor_scalar(ctx_q[:], c_ps[:], SX, None, ALU.mult)
            dst = cstk_d[:, bass.ds(t4, BL)].rearrange("(c p) b -> p c b", p=128)
            nc.sync.dma_start(dst, ctxT_sb[:])

        with tc.For_i(0, _nsteps * BL, UNROLL * BL,
                      hint_engines=(ET.PE, ET.Activation, ET.DVE, ET.SP)) as t4:
            for s in range(UNROLL):
                step_body(t4 + s * BL)

        # ============ READOUT ============
        inner.close()
        post_sb = ctx.enter_context(tc.tile_pool(name="post_sb", bufs=1))
        post_st = ctx.enter_context(tc.tile_pool(name="post_st", bufs=2))
        post_ps = ctx.enter_context(tc.tile_pool(name="post_ps", bufs=2,
                                                 space="PSUM"))
        trash_holder.append(post_ps.tile([128, 128], BF16, name="trash_post"))

        xro = []
        for k in range(HC):
            tl = post_sb.tile([128, NS], BF16, name=f"xh{k}", tag=f"xh{k}")
            nc.sync.dma_start(tl[:], hstk_d[k * 128:(k + 1) * 128, :])
            xro.append(tl)
        for k in range(EC):
            tl = post_sb.tile([128, NS], BF16, name=f"xe{k}", tag=f"xe{k}")
            nc.sync.dma_start(tl[:], embT_d[k * 128:(k + 1) * 128, :])
            xro.append(tl)
        for k in range(DC):
            tl = post_sb.tile([128, NS], BF16, name=f"xc{k}", tag=f"xc{k}")
            nc.sync.dma_start(tl[:], cstk_d[k * 128:(k + 1) * 128, :])
            xro.append(tl)
        W_roe = [post_sb.tile([128, RO // 2], BF16, name=f"wre{k}", tag=f"wre{k}")
                 for k in range(XROC)]
        W_roo = [post_sb.tile([128, RO // 2], BF16, name=f"wro{k}", tag=f"wro{k}")
                 for k in range(XROC)]
        for k in range(XROC):
            nc.sync.dma_start(W_roe[k][:], W_roT_e_d[k * 128:(k + 1) * 128, :])
            nc.sync.dma_start(W_roo[k][:], W_roT_o_d[k * 128:(k + 1) * 128, :])
        b_ro_e = post_sb.tile([128, ROC], F32)
        nc.sync.dma_start(b_ro_e[:], b_ro_e_d[:, :])
        b_ro_o = post_sb.tile([128, ROC], F32)
        nc.sync.dma_start(b_ro_o[:], b_ro_o_d[:, :])
        b_out_col = post_sb.tile([128, VC], F32)
        nc.sync.dma_start(b_out_col[:], b_out_d[:, :])
        pe_touch(xro[0][:, 0:128])
        pe_touch(W_roe[0][:, 0:128])
        pe_touch(W_roo[0][:, 0:128])

        maxo = []
        for oc in range(ROC):
            Re = post_ps.tile([128, NS], F32, name="Re", tag="Re")
            for k in range(XROC):
                nc.tensor.matmul(Re[:], W_roe[k][:, oc * 128:(oc + 1) * 128],
                                 xro[k][:], start=(k == 0), stop=(k == XROC - 1))
            t1 = post_st.tile([128, NS], F32, name="t1", tag="t1")
            nc.scalar.activation(t1[:], Re[:], AF.Identity,
                                 bias=b_ro_e[:, oc:oc + 1], scale=1.0)
            Ro = post_ps.tile([128, NS], F32, name="Ro", tag="Re")
            for k in range(XROC):
                nc.tensor.matmul(Ro[:], W_roo[k][:, oc * 128:(oc + 1) * 128],
                                 xro[k][:], start=(k == 0), stop=(k == XROC - 1))
            t2 = post_st.tile([128, NS], F32, name="t2", tag="t2")
            nc.scalar.activation(t2[:], Ro[:], AF.Identity,
                                 bias=b_ro_o[:, oc:oc + 1], scale=1.0)
            mo = post_sb.tile([128, NS], BF16, name=f"mo{oc}", tag=f"mo{oc}")
            nc.vector.tensor_tensor(mo[:], t1[:], t2[:], ALU.max)
            maxo.append(mo)

        wo_pool = ctx.enter_context(tc.tile_pool(name="wo", bufs=6))
        first = True
        for vc in range(VC):
            wo = [wo_pool.tile([128, 128], BF16, name=f"wo{vc}_{k}", tag=f"wok{k}")
                  for k in range(ROC)]
            for k in range(ROC):
                nc.sync.dma_start(wo[k][:],
                                  W_outT_d[k * 128:(k + 1) * 128,
                                           vc * 128:(vc + 1) * 128])
            if first:
                pe_touch(wo[0][:, 0:128])
                pe_touch(maxo[0][:, 0:128])
                first = False
            L = post_ps.tile([128, NS], F32, name="L", tag="L")
            for k in range(ROC):
                nc.tensor.matmul(L[:], wo[k][:], maxo[k][:],
                                 start=(k == 0), stop=(k == ROC - 1))
            lo = post_st.tile([128, NS], F32, name="lo", tag="lo")
            nc.scalar.activation(lo[:], L[:], AF.Identity,
                                 bias=b_out_col[:, vc:vc + 1], scale=1.0)
            nc.sync.dma_start(out_d[vc * 128:(vc + 1) * 128, :], lo[:])

    return nc


def check_waits(nc, cap_note=""):
    bad = []
    for fn in nc.m.functions:
        for bb in fn.blocks:
            for inst in bb.instructions:
                c = inst.concise()
                nw = c.count("wait:")
                eng = c.split()[0] if c.split() else "?"
                if nw >= 2 and eng in ("PE", "ACT", "DVE", "PL"):
                    bad.append((nw, c[:180]))
    for nw, c in bad:
        print("WAITS", nw, c)
    return bad


def _prep_core(inputs, order, Tp, core):
    enc = np.asarray(inputs["encoder_outputs"], np.float32)
    labels = np.asarray(inputs["labels"])
    lens = np.asarray(inputs["enc_seq_len"], np.int64)
    embed = np.asarray(inputs["embed"], np.float32)

    bidx = [int(order[j * NCORE + core]) for j in range(BL)]
    m = {}
    for j in range(BL):
        b = bidx[j]
        ep = np.zeros((Tp[j], D), np.float32)
        ep[:T] = enc[b, :Tp[j] if Tp[j] <= T else T]
        m[f"enc_td{j}"] = _bf(ep)
        m[f"encT{j}"] = _bf(ep.T)
    emb = np.zeros((BL, S, E), np.float32)
    for j in range(BL):
        b = bidx[j]
        emb[j, 1:] = embed[labels[b, :S - 1].astype(np.int64)]
    embT = emb.transpose(2, 1, 0).reshape(E, NS)
    m["embT"] = _bf(embT)
    mask01 = np.zeros((BL, 1024), np.float32)
    for j in range(BL):
        mask01[j, :int(lens[bidx[j]])] = 1.0
    m["mask01"] = _bf(mask01)
    return m, bidx


def kernel(**inputs):
    lens = np.asarray(inputs["enc_seq_len"], np.int64)
    order = np.argsort(-lens, kind="stable")
    Tp = []
    for j in range(BL):
        mx = max(int(lens[order[j * NCORE + i]]) for i in range(NCORE))
        Tp.append(min(1024, ((mx + 127) // 128) * 128))

    perm = _gate_perm()
    W_ih = np.asarray(inputs["W_ih"], np.float32)[perm]
    W_hh = np.asarray(inputs["W_hh"], np.float32)[perm]
    b_sum = (np.asarray(inputs["b_ih"], np.float32)
             + np.asarray(inputs["b_hh"], np.float32))[perm]
    wfb = np.asarray(inputs["W_fb"], np.float32)[:, 0]
    wfb_safe = np.where(wfb >= 0, np.maximum(wfb, 1e-3),
                        np.minimum(wfb, -1e-3))
    shared = {
        "W_combT": _w8(np.concatenate([W_ih[:, E:].T, W_hh.T], 0)),
        "W_ih_embT": _bf(W_ih[:, :E].T),
        "W_encT": _bf(np.asarray(inputs["W_enc"], np.float32).T
                      / wfb_safe[None, :]),
        "W_sT": _w8(np.asarray(inputs["W_s"], np.float32).T),
        "wfert_col": _bf(np.asarray(inputs["W_fert"],
                                    np.float32).reshape(DC, 128).T),
        "vT_col": _bf(np.asarray(inputs["v_att"], np.float32).reshape(AC, 128).T),
        "wfb_colf": np.ascontiguousarray(
            wfb_safe.reshape(AC, 128).T.astype(np.float32)),
        "b_enc_col": np.ascontiguousarray(
            (np.asarray(inputs["b_enc"], np.float32) / wfb_safe)
            .reshape(AC, 128).T),
        "b_comb": np.ascontiguousarray(
            (b_sum * SG).reshape(GC, 128).T),
        "W_roT_e": _bf(np.asarray(inputs["W_ro"], np.float32)[0::2].T),
        "W_roT_o": _bf(np.asarray(inputs["W_ro"], np.float32)[1::2].T),
        "b_ro_e": np.ascontiguousarray(
            np.asarray(inputs["b_ro"], np.float32)[0::2].reshape(ROC, 128).T),
        "b_ro_o": np.ascontiguousarray(
            np.asarray(inputs["b_ro"], np.float32)[1::2].reshape(ROC, 128).T),
        "W_outT": _bf(np.asarray(inputs["W_out"], np.float32).T),
        "b_out_col": np.ascontiguousarray(
            np.asarray(inputs["b_out"], np.float32).reshape(VC, 128).T),
    }

    in_maps = []
    bidx_all = []
    for c in range(NCORE):
        m, bidx = _prep_core(inputs, order, Tp, c)
        m.update(shared)
        in_maps.append(m)
        bidx_all.append(bidx)

    nc = build_nc(Tp)
    nc.finalize()
    from concourse.bass_utils import run_bass_kernel_spmd
    trace = bool(os.environ.get("BASS_KERNEL_TRACE"))
    res = run_bass_kernel_spmd(nc, in_maps, core_ids=list(range(NCORE)),
                               trace=trace)
    global LAST_EXEC_NS, LAST_OUTS, LAST_META
    LAST_EXEC_NS = res.exec_time_ns
    outs = res.results
    LAST_OUTS = outs
    LAST_META = (order, Tp, bidx_all)

    logits = np.zeros((B, S, V), np.float32)
    for c in range(NCORE):
        o = outs[c]["out"].reshape(V, S, BL)
        for j in range(BL):
            logits[bidx_all[c][j]] = o[:, :, j].T
    return logits


if __name__ == "__main__":
    nc = build_nc([1024, 896, 768, 640])
    bad = check_waits(nc)
    print(f"{len(bad)} instructions with >=2 waits")
